# revision 1
# baseline (speedup 1.0000x reference)
"""Trainium2 Bass kernel for nn_MistralMoLoraLayer (MoE-routed LoRA FFN).

Strategy: data-parallel over tokens (8 cores x 256 tokens), base FFN weights
replicated, all-expert LoRA replicated. The per-(batch,slot) softmax over the
sequence axis needs global denominators -> tiny [2,8] AllReduce.

Per-core math (all tiles [h/er/d partitions, tokens free]):
  router: logits = x @ gate_w.T; top-2 (value,index) per token; exp; AR of
          per-batch-slot sums; weights w_j = exp_j / denom[batch, j]
  A-proj: UA/GA [E*R=128, t] = stacked up_A/gate_A @ x.T   (one K=128 chain)
  slot-mask trick: Ut_j = UA * M_j where M_j[e*R+r, t] = (sel_j(t)==e);
          lo_up_j[h,t] = (stacked up_B) @ Ut_j  == up_B[sel_j(t)] @ u_{sel_j(t)}
  h_j = silu(U + lo_up_j) * (G + lo_gate_j); ch_j = c_j * h_j
  mixed = ch_0 + ch_1
  v_j[er,t] = (stacked down_A) @ ch_j  (accumulated over h), masked by M_j
  outT[d,t] = w_down-chain @ mixed + (stacked down_B) @ v_0 + ... @ v_1
"""

import numpy as np

# problem constants (hardcoded; kernel.py must be self-contained)
B, S, D, H, E, R, TOPK = 2, 1024, 2048, 5632, 8, 16, 2
ALPHA = 2.0
T = B * S
NCORES = 8
TC = T // NCORES           # 256 tokens per core
KT = D // 128              # 16 k-tiles over D
HT = H // 128              # 44 h-tiles
DT = D // 128              # 16 d-tiles
ER = E * R                 # 128

MM_MODE = "hyb"            # "f32" | "f32r" | "bf16" | "hyb"
DEBUG_TAPS = False         # add intermediate-tensor outputs for debugging
SKIP_AR = False            # replace AllReduce with local copy (for TimelineSim)

_cache = {}


def _np_sd():
    import ml_dtypes
    return np.dtype(ml_dtypes.bfloat16) if MM_MODE == "bf16" else np.dtype(np.float32)


def _build():
    import concourse.bacc as bacc
    import concourse.bass as bass
    import concourse.mybir as mybir
    import concourse.tile as tile
    from concourse.masks import make_identity

    f32 = mybir.dt.float32
    bf16 = mybir.dt.bfloat16
    SD = bf16 if MM_MODE == "bf16" else f32
    WUG = bf16 if MM_MODE in ("bf16", "hyb") else f32  # up/gate weights + x
    AL = mybir.AluOpType
    AF = mybir.ActivationFunctionType

    def mm(ap):
        # matmul operand dtype override for f32r-path tensors
        if MM_MODE in ("f32r", "hyb"):
            return ap.bitcast(mybir.dt.float32r)
        return ap

    mo = mm  # producer outputs feeding f32r matmuls must also be f32r-typed

    def mug(ap):
        # up/gate-path operands: true bf16 in hyb/bf16, f32r in f32r mode
        if MM_MODE == "f32r":
            return ap.bitcast(mybir.dt.float32r)
        return ap

    nc = bacc.Bacc("TRN2", target_bir_lowering=False, debug=False,
                   num_devices=NCORES)

    # ---- DRAM I/O ----
    d_xT = nc.dram_tensor("xT", [D, TC], SD, kind="ExternalInput").ap()
    if MM_MODE == "bf16":
        d_xTr = nc.dram_tensor("xTr", [D, TC], f32, kind="ExternalInput").ap()
    else:
        d_xTr = d_xT
    d_gw = nc.dram_tensor("gw", [128, KT * E], f32, kind="ExternalInput").ap()
    d_wu = nc.dram_tensor("wu", [HT, 128, KT * 128], WUG, kind="ExternalInput").ap()
    d_wg = nc.dram_tensor("wg", [HT, 128, KT * 128], WUG, kind="ExternalInput").ap()
    d_wd = nc.dram_tensor("wd", [DT, 128, HT * 128], SD, kind="ExternalInput").ap()
    d_A = nc.dram_tensor("Ah", [128, KT * 2 * ER], SD, kind="ExternalInput").ap()
    d_uB = nc.dram_tensor("uB", [HT, 128, 128], SD, kind="ExternalInput").ap()
    d_gB = nc.dram_tensor("gB", [HT, 128, 128], SD, kind="ExternalInput").ap()
    d_dA = nc.dram_tensor("dA", [HT, 128, ER], SD, kind="ExternalInput").ap()
    d_dB = nc.dram_tensor("dB", [128, D], SD, kind="ExternalInput").ap()
    d_eid = nc.dram_tensor("eid", [128, 1], f32, kind="ExternalInput").ap()
    d_i8m = nc.dram_tensor("i8m", [128, E], f32, kind="ExternalInput").ap()
    d_bsr = nc.dram_tensor("bsr", [1, 2], f32, kind="ExternalInput").ap()
    d_bsc = nc.dram_tensor("bsc", [2, 1], f32, kind="ExternalInput").ap()
    d_sel2 = nc.dram_tensor("sel2", [2, 256], f32, kind="ExternalInput").ap()
    d_out = nc.dram_tensor("outT", [D, TC], f32, kind="ExternalOutput").ap()

    with tile.TileContext(nc) as tc:
        import contextlib
        ctx = contextlib.ExitStack()
        with ctx:
            cpool = ctx.enter_context(tc.tile_pool(name="const", bufs=1))
            wpool = ctx.enter_context(tc.tile_pool(name="wstream", bufs=2))
            bpool = ctx.enter_context(tc.tile_pool(name="bstream", bufs=3))
            spool = ctx.enter_context(tc.tile_pool(name="work", bufs=2))
            pspool = ctx.enter_context(
                tc.tile_pool(name="ps", bufs=1, space="PSUM"))
            drpool = ctx.enter_context(
                tc.tile_pool(name="dram", bufs=1, space="DRAM"))

            # ---- constants / resident tiles ----
            xT_sb = cpool.tile([128, KT * TC], SD, name="xT_sb")
            for k in range(KT):
                nc.sync.dma_start(out=mo(xT_sb[:, k * TC:(k + 1) * TC]),
                                  in_=mo(d_xT[k * 128:(k + 1) * 128, :]))
            if MM_MODE == "bf16":
                xTr_sb = cpool.tile([128, KT * TC], f32, name="xTr_sb")
                for k in range(KT):
                    nc.sync.dma_start(out=xTr_sb[:, k * TC:(k + 1) * TC],
                                      in_=d_xTr[k * 128:(k + 1) * 128, :])
            elif MM_MODE in ("f32r", "hyb"):
                xTr_sb = xT_sb.bitcast(f32)   # same bits, f32 view for router
            else:
                xTr_sb = xT_sb
            if MM_MODE == "hyb":
                # bf16 copy of x for the up/gate base GEMMs (gpsimd casts)
                xTb = cpool.tile([128, KT * TC], bf16, name="xTb")
                for k in range(KT):
                    nc.gpsimd.dma_start(out=xTb[:, k * TC:(k + 1) * TC],
                                        in_=d_xT[k * 128:(k + 1) * 128, :])
            else:
                xTb = xT_sb
            A_sb = cpool.tile([128, KT * 2 * ER], SD, name="A_sb")
            nc.sync.dma_start(out=mo(A_sb[:]), in_=mo(d_A[:]))
            dB_sb = cpool.tile([128, D], SD, name="dB_sb")
            nc.sync.dma_start(out=mo(dB_sb[:]), in_=mo(d_dB[:]))
            gw_sb = cpool.tile([128, KT * E], f32, name="gw_sb")
            nc.sync.dma_start(out=gw_sb[:], in_=d_gw[:])
            eid_sb = cpool.tile([128, 1], f32, name="eid_sb")
            nc.sync.dma_start(out=eid_sb[:], in_=d_eid[:])
            i8m_sb = cpool.tile([128, E], f32, name="i8m_sb")
            nc.sync.dma_start(out=i8m_sb[:], in_=d_i8m[:])
            bsr_sb = cpool.tile([1, 2], f32, name="bsr_sb")
            nc.sync.dma_start(out=bsr_sb[:], in_=d_bsr[:])
            bsc_sb = cpool.tile([2, 1], f32, name="bsc_sb")
            nc.sync.dma_start(out=bsc_sb[:], in_=d_bsc[:])
            sel2_sb = cpool.tile([2, 256], f32, name="sel2_sb")
            nc.sync.dma_start(out=sel2_sb[:], in_=d_sel2[:])

            ident = cpool.tile([128, 128], f32, name="ident")
            make_identity(nc, ident)
            ones_row = cpool.tile([1, 128], f32, name="ones_row")
            nc.vector.memset(ones_row, 1.0)
            ones_col = cpool.tile([128, 1], f32, name="ones_col")
            nc.vector.memset(ones_col, 1.0)

            mixed = cpool.tile([128, HT * TC], SD, name="mixed")
            ev_rows = cpool.tile([2, TC], f32, name="ev_rows")
            s_rows = cpool.tile([2, TC], f32, name="s_rows")
            crows = cpool.tile([2, TC], f32, name="crows")
            cb = cpool.tile([128, 2 * TC], SD, name="cb")
            Mj = cpool.tile([128, 2 * TC], SD, name="Mj")
            UA = cpool.tile([128, TC], SD, name="UA")
            GA = cpool.tile([128, TC], SD, name="GA")
            Ut = cpool.tile([128, 2 * TC], SD, name="Ut")
            Gt = cpool.tile([128, 2 * TC], SD, name="Gt")
            vt = cpool.tile([128, 2 * TC], SD, name="vt")

            # ---- phase 1: router ----
            den_parts = cpool.tile([1, 4], f32, name="den_parts")
            for tt in range(2):
                psL = pspool.tile([128, TC], f32, tag="ps_small", name="psL")
                for k in range(KT):
                    nc.tensor.matmul(
                        psL[:, 0:E],
                        xTr_sb[:, k * TC + tt * 128: k * TC + tt * 128 + 128],
                        gw_sb[:, k * E:(k + 1) * E],
                        start=(k == 0), stop=(k == KT - 1))
                L = spool.tile([128, E], f32, tag="L")
                nc.vector.tensor_copy(L[:], psL[:, 0:E])
                mx1 = spool.tile([128, 1], f32, tag="mx1")
                nc.vector.tensor_reduce(mx1[:], L[:], mybir.AxisListType.X, AL.max)
                msk = spool.tile([128, E], f32, tag="msk")
                nc.vector.tensor_scalar(msk[:], L[:], mx1[:], None, AL.is_equal)
                mi = spool.tile([128, E], f32, tag="mi")
                nc.vector.tensor_tensor(mi[:], msk[:], i8m_sb[:], AL.mult)
                svals = spool.tile([128, 2], f32, tag="svals")
                nc.vector.tensor_reduce(svals[:, 0:1], mi[:],
                                        mybir.AxisListType.X, AL.max)
                evals = spool.tile([128, 2], f32, tag="evals")
                nc.scalar.activation(evals[:, 0:1], mx1[:], AF.Exp)
                # mask out slot-0 winner, find second max
                big = spool.tile([128, E], f32, tag="big")
                nc.vector.tensor_scalar(big[:], msk[:], 1e30, None, AL.mult)
                L2 = spool.tile([128, E], f32, tag="L2")
                nc.vector.tensor_tensor(L2[:], L[:], big[:], AL.subtract)
                mx2 = spool.tile([128, 1], f32, tag="mx2")
                nc.vector.tensor_reduce(mx2[:], L2[:], mybir.AxisListType.X, AL.max)
                msk2 = spool.tile([128, E], f32, tag="msk2")
                nc.vector.tensor_scalar(msk2[:], L2[:], mx2[:], None, AL.is_equal)
                mi2 = spool.tile([128, E], f32, tag="mi2")
                nc.vector.tensor_tensor(mi2[:], msk2[:], i8m_sb[:], AL.mult)
                nc.vector.tensor_reduce(svals[:, 1:2], mi2[:],
                                        mybir.AxisListType.X, AL.max)
                nc.scalar.activation(evals[:, 1:2], mx2[:], AF.Exp)
                # per-tile partial denominators: [1,2] = ones.T @ evals
                psd = pspool.tile([1, 2], f32, tag="ps_small", name="psd")
                nc.tensor.matmul(psd[:], ones_col[:], evals[:],
                                 start=True, stop=True)
                nc.vector.tensor_copy(den_parts[:, tt * 2:(tt + 1) * 2], psd[:])
                # transpose evals/svals -> rows
                psT = pspool.tile([2, 128], f32, tag="ps_small", name="psT")
                nc.tensor.transpose(psT[:], evals[:], ident[:])
                nc.vector.tensor_copy(ev_rows[:, tt * 128:(tt + 1) * 128], psT[:])
                psT2 = pspool.tile([2, 128], f32, tag="ps_small", name="psT2")
                nc.tensor.transpose(psT2[:], svals[:], ident[:])
                nc.vector.tensor_copy(s_rows[:, tt * 128:(tt + 1) * 128], psT2[:])

            # combine partials, AllReduce [2,8] (row b = batch, cols 0:2 used)
            drow = cpool.tile([1, 2], f32, name="drow")
            nc.vector.tensor_tensor(drow[:], den_parts[:, 0:2],
                                    den_parts[:, 2:4], AL.add)
            ar_sb = cpool.tile([2, 8], f32, name="ar_sb")
            nc.vector.memset(ar_sb, 0.0)
            psAR = pspool.tile([2, 2], f32, tag="ps_small", name="psAR")
            nc.tensor.matmul(psAR[:], bsr_sb[:], drow[:], start=True, stop=True)
            nc.vector.tensor_copy(ar_sb[:, 0:2], psAR[:])
            ar_in = drpool.tile([2, 8], f32, name="ar_in")
            ar_out = drpool.tile([2, 8], f32, name="ar_out", addr_space="Shared")
            nc.gpsimd.dma_start(out=ar_in[:], in_=ar_sb[:])
            if SKIP_AR:
                nc.gpsimd.dma_start(out=ar_out[:], in_=ar_in[:])
            else:
                nc.gpsimd.collective_compute(
                    "AllReduce", AL.add,
                    replica_groups=[list(range(NCORES))],
                    ins=[ar_in.opt()], outs=[ar_out.opt()])
            den2 = cpool.tile([2, 8], f32, name="den2")
            nc.gpsimd.dma_start(out=den2[:], in_=ar_out[:])
            # select this core's batch row -> [2(slots),1], reciprocal
            psDC = pspool.tile([2, 1], f32, tag="ps_small", name="psDC")
            nc.tensor.matmul(psDC[:], den2[:, 0:2], bsc_sb[:],
                             start=True, stop=True)
            rcp = cpool.tile([2, 1], f32, name="rcp")
            nc.vector.reciprocal(rcp[:], psDC[:])
            # normalized routing weights as rows [2, TC]
            nc.vector.tensor_scalar(crows[:], ev_rows[:], rcp[:], None, AL.mult)

            # broadcast slot rows along partitions via K=2 matmul with a
            # row-selector constant (sel2[:, j*128:(j+1)*128] has row j = 1)
            for j in range(2):
                psB = pspool.tile([128, TC], f32, tag="ps_small", name="psB")
                nc.tensor.matmul(psB[:], sel2_sb[:, j * 128:(j + 1) * 128],
                                 crows[:], start=True, stop=True)
                nc.vector.tensor_copy(cb[:, j * TC:(j + 1) * TC], psB[:])
                psM = pspool.tile([128, TC], f32, tag="ps_small", name="psM")
                nc.tensor.matmul(psM[:], sel2_sb[:, j * 128:(j + 1) * 128],
                                 s_rows[:], start=True, stop=True)
                nc.vector.tensor_scalar(Mj[:, j * TC:(j + 1) * TC], psM[:],
                                        eid_sb[:], None, AL.is_equal)

            # ---- phase 3: stacked A-projections ----
            psUA = pspool.tile([128, TC], f32, tag="psUG", bufs=2, name="psUA")
            for k in range(KT):
                nc.tensor.matmul(psUA[:],
                                 mm(A_sb[:, k * 2 * ER: k * 2 * ER + ER]),
                                 mm(xT_sb[:, k * TC:(k + 1) * TC]),
                                 start=(k == 0), stop=(k == KT - 1))
            nc.vector.tensor_copy(UA[:], psUA[:])
            psGA = pspool.tile([128, TC], f32, tag="psUG", bufs=2, name="psGA")
            for k in range(KT):
                nc.tensor.matmul(psGA[:],
                                 mm(A_sb[:, k * 2 * ER + ER:(k + 1) * 2 * ER]),
                                 mm(xT_sb[:, k * TC:(k + 1) * TC]),
                                 start=(k == 0), stop=(k == KT - 1))
            nc.vector.tensor_copy(GA[:], psGA[:])
            for j in range(2):
                nc.vector.tensor_tensor(mo(Ut[:, j * TC:(j + 1) * TC]), UA[:],
                                        Mj[:, j * TC:(j + 1) * TC], AL.mult)
                nc.vector.tensor_tensor(mo(Gt[:, j * TC:(j + 1) * TC]), GA[:],
                                        Mj[:, j * TC:(j + 1) * TC], AL.mult)

            # ---- phases 2+5+6: h-tile loop ----
            psV = pspool.tile([128, 2 * TC], f32, tag="psV", name="psV")
            KH = KT // 2 * 128          # half of the k columns (1024)
            QH = HT // 4 * 128          # quarter of the h columns (1408)
            wd_pre = {}                 # (di, q) -> prefetched tile
            pend_v = None               # delayed psV matmul (dA_t, ch_pair)

            def load_wd(di, q):
                t = wpool.tile([128, QH], SD, tag="wd", bufs=6, name="wd_t")
                nc.sync.dma_start(
                    out=mo(t[:]), in_=mo(d_wd[di][:, q * QH:(q + 1) * QH]))
                return t

            for i in range(HT):
                if 4 <= i < 10:
                    k6 = i - 4          # prefetch 6 wd quarters mid-loop
                    wd_pre[(k6 // 4, k6 % 4)] = load_wd(k6 // 4, k6 % 4)
                wu_h = []
                wg_h = []
                for hf in range(2):
                    wu_t = wpool.tile([128, KH], WUG, tag="wu", bufs=4)
                    nc.sync.dma_start(
                        out=mug(wu_t[:]),
                        in_=mug(d_wu[i][:, hf * KH:(hf + 1) * KH]))
                    wu_h.append(wu_t)
                    wg_t = wpool.tile([128, KH], WUG, tag="wg", bufs=4)
                    nc.sync.dma_start(
                        out=mug(wg_t[:]),
                        in_=mug(d_wg[i][:, hf * KH:(hf + 1) * KH]))
                    wg_h.append(wg_t)
                uB_t = bpool.tile([128, 128], SD, tag="uB")
                nc.sync.dma_start(out=mo(uB_t[:]), in_=mo(d_uB[i]))
                gB_t = bpool.tile([128, 128], SD, tag="gB")
                nc.sync.dma_start(out=mo(gB_t[:]), in_=mo(d_gB[i]))
                dA_t = bpool.tile([128, ER], SD, tag="dA")
                nc.sync.dma_start(out=mo(dA_t[:]), in_=mo(d_dA[i]))

                psUG = pspool.tile([128, 2 * TC], f32, tag="psUG", bufs=2,
                                   name="psUG")
                for k in range(KT):
                    w = wu_h[k // 8][:, (k % 8) * 128:(k % 8 + 1) * 128]
                    nc.tensor.matmul(psUG[:, 0:TC], mug(w),
                                     mug(xTb[:, k * TC:(k + 1) * TC]),
                                     start=(k == 0), stop=(k == KT - 1))
                for k in range(KT):
                    w = wg_h[k // 8][:, (k % 8) * 128:(k % 8 + 1) * 128]
                    nc.tensor.matmul(psUG[:, TC:2 * TC], mug(w),
                                     mug(xTb[:, k * TC:(k + 1) * TC]),
                                     start=(k == 0), stop=(k == KT - 1))
                if pend_v is not None:
                    pv_dA, pv_ch = pend_v
                    nc.tensor.matmul(psV[:], mm(pv_dA[:]), mm(pv_ch[:]),
                                     start=(i == 1), stop=False,
                                     skip_group_check=True)
                U_sb = spool.tile([128, TC], SD, tag="U_sb")
                nc.scalar.copy(U_sb[:], psUG[:, 0:TC])
                G_sb = spool.tile([128, TC], SD, tag="G_sb")
                nc.scalar.copy(G_sb[:], psUG[:, TC:2 * TC])

                psLO = pspool.tile([128, 4 * TC], f32, tag="psLO", bufs=2,
                                   name="psLO")
                # both slots' c*h in ONE tile so the down_A contraction is a
                # single [128,512] matmul per h-tile (avoids the whole-bank
                # has_written clear from a second start=True in the same bank)
                ch_pair = spool.tile([128, 2 * TC], SD, tag="chp", bufs=3)
                for j in range(2):
                    nc.tensor.matmul(psLO[:, (2 * j) * TC:(2 * j + 1) * TC],
                                     mm(uB_t[:]),
                                     mm(Ut[:, j * TC:(j + 1) * TC]),
                                     start=True, stop=True)
                    nc.tensor.matmul(psLO[:, (2 * j + 1) * TC:(2 * j + 2) * TC],
                                     mm(gB_t[:]),
                                     mm(Gt[:, j * TC:(j + 1) * TC]),
                                     start=True, stop=True)
                    tu = spool.tile([128, TC], SD, tag="tu")
                    nc.vector.tensor_tensor(
                        tu[:], U_sb[:], psLO[:, (2 * j) * TC:(2 * j + 1) * TC],
                        AL.add)
                    su = spool.tile([128, TC], SD, tag="su")
                    nc.scalar.activation(su[:], tu[:], AF.Silu)
                    tg = spool.tile([128, TC], SD, tag="tg")
                    nc.vector.tensor_tensor(
                        tg[:], G_sb[:],
                        psLO[:, (2 * j + 1) * TC:(2 * j + 2) * TC], AL.add)
                    hh = spool.tile([128, TC], SD, tag="hh")
                    nc.vector.tensor_tensor(hh[:], su[:], tg[:], AL.mult)
                    nc.vector.tensor_tensor(mo(ch_pair[:, j * TC:(j + 1) * TC]),
                                            hh[:],
                                            cb[:, j * TC:(j + 1) * TC], AL.mult)
                # psV matmul for tile i-1 is emitted AFTER tile i's base
                # matmuls: keeps the in-order PE queue from stalling on the
                # DVE chain that produces ch_pair (head-of-line blocking)
                nc.vector.tensor_tensor(mo(mixed[:, i * TC:(i + 1) * TC]),
                                        ch_pair[:, 0:TC], ch_pair[:, TC:2 * TC],
                                        AL.add)
                pend_v = (dA_t, ch_pair)

            pv_dA, pv_ch = pend_v
            nc.tensor.matmul(psV[:], mm(pv_dA[:]), mm(pv_ch[:]),
                             start=False, stop=True, skip_group_check=True)
            # masked v
            for j in range(2):
                nc.vector.tensor_tensor(mo(vt[:, j * TC:(j + 1) * TC]),
                                        psV[:, j * TC:(j + 1) * TC],
                                        Mj[:, j * TC:(j + 1) * TC], AL.mult)

            if DEBUG_TAPS:
                for nm, tl in [("crows", crows), ("srows", s_rows),
                               ("cb", cb), ("Mj", Mj), ("UA", UA),
                               ("GA", GA), ("vt", vt),
                               ("mixed0", mixed[:, 0:TC]),
                               ("mixed7", mixed[:, 7 * TC:8 * TC])]:
                    shp = [tl.shape[0], tl.shape[-1]]
                    dbg = nc.dram_tensor(f"dbg_{nm}", shp, f32,
                                         kind="ExternalOutput").ap()
                    nc.sync.dma_start(out=dbg[:], in_=tl[:])

            # ---- phase 7: down GEMM + LoRA-down ----
            for di in range(DT):
                wd_q = [wd_pre.get((di, q)) or load_wd(di, q)
                        for q in range(4)]
                psO = pspool.tile([128, TC], f32, tag="psUG", bufs=2, name="psO")
                for hk in range(HT):
                    w = wd_q[hk // 11][:, (hk % 11) * 128:(hk % 11 + 1) * 128]
                    nc.tensor.matmul(psO[:], mm(w),
                                     mm(mixed[:, hk * TC:(hk + 1) * TC]),
                                     start=(hk == 0), stop=False,
                                     skip_group_check=True)
                nc.tensor.matmul(psO[:], mm(dB_sb[:, di * 128:(di + 1) * 128]),
                                 mm(vt[:, 0:TC]), start=False, stop=False,
                                 skip_group_check=True)
                nc.tensor.matmul(psO[:], mm(dB_sb[:, di * 128:(di + 1) * 128]),
                                 mm(vt[:, TC:2 * TC]), start=False, stop=True,
                                 skip_group_check=True)
                o_sb = spool.tile([128, TC], f32, tag="o_sb")
                nc.scalar.copy(o_sb[:], psO[:])
                nc.sync.dma_start(out=d_out[di * 128:(di + 1) * 128, :],
                                  in_=o_sb[:])

    nc.compile()
    return nc


def _prep_shared(inputs):
    """Host-side layout prep of weight tensors (shared across cores)."""
    import ml_dtypes
    sd = _np_sd()
    sd_ug = (np.dtype(ml_dtypes.bfloat16) if MM_MODE in ("bf16", "hyb")
             else np.dtype(np.float32))
    f32 = np.float32

    def c(a, dt):
        return np.ascontiguousarray(a.astype(dt, copy=False))

    w_up, w_gate, w_down = inputs["w_up"], inputs["w_gate"], inputs["w_down"]
    wu = c(w_up.reshape(HT, 128, KT, 128).transpose(0, 3, 2, 1)
           .reshape(HT, 128, KT * 128), sd_ug)
    wg = c(w_gate.reshape(HT, 128, KT, 128).transpose(0, 3, 2, 1)
           .reshape(HT, 128, KT * 128), sd_ug)
    wd = c(w_down.reshape(DT, 128, HT, 128).transpose(0, 3, 2, 1)
           .reshape(DT, 128, HT * 128), sd)

    A_stack = np.concatenate([
        inputs["up_A"].reshape(ER, D),
        inputs["gate_A"].reshape(ER, D)], axis=0)          # [2*ER, D]
    # Ah[p, k*2ER + m] = A_stack[m, k*128+p]
    Ah = c(A_stack.reshape(2 * ER, KT, 128).transpose(2, 1, 0)
           .reshape(128, KT * 2 * ER), sd)

    up_B_all = (inputs["up_B"].transpose(0, 2, 1).reshape(ER, H)
                * ALPHA).astype(f32)
    gate_B_all = (inputs["gate_B"].transpose(0, 2, 1).reshape(ER, H)
                  * ALPHA).astype(f32)
    uB = c(up_B_all.reshape(ER, HT, 128).transpose(1, 0, 2), sd)
    gB = c(gate_B_all.reshape(ER, HT, 128).transpose(1, 0, 2), sd)

    down_A_all = inputs["down_A"].reshape(ER, H).astype(f32)
    dA = c(down_A_all.T.reshape(HT, 128, ER), sd)
    down_B_all = (inputs["down_B"].transpose(0, 2, 1).reshape(ER, D)
                  * ALPHA).astype(f32)
    dB = c(down_B_all, sd)

    gate_wT = inputs["gate_w"].T.astype(f32)               # [D, E]
    gw = c(gate_wT.reshape(KT, 128, E).transpose(1, 0, 2)
           .reshape(128, KT * E), f32)

    eid = (8.0 - (np.arange(128) // R)).astype(f32).reshape(128, 1)
    i8m = np.tile((8.0 - np.arange(E)).astype(f32), (128, 1))
    sel2 = np.zeros((2, 256), f32)
    sel2[0, 0:128] = 1.0
    sel2[1, 128:256] = 1.0

    return dict(wu=wu, wg=wg, wd=wd, Ah=Ah, uB=uB, gB=gB, dA=dA, dB=dB,
                gw=gw, eid=eid, i8m=i8m, sel2=sel2)


def kernel(**inputs):
    from concourse.bass_utils import run_bass_kernel_spmd

    inputs = {k: np.asarray(v) for k, v in inputs.items()}
    if "nc" not in _cache:
        _cache["nc"] = _build()
    nc = _cache["nc"]

    shared = _prep_shared(inputs)
    sd = _np_sd()
    x = inputs["x"].astype(np.float32)
    xt = x.reshape(T, D)

    in_maps = []
    for cix in range(NCORES):
        xc = xt[cix * TC:(cix + 1) * TC]                   # [TC, D]
        xT = np.ascontiguousarray(xc.T)                    # [D, TC] f32
        b = (cix * TC) // S
        bsr = np.zeros((1, 2), np.float32); bsr[0, b] = 1.0
        bsc = np.zeros((2, 1), np.float32); bsc[b, 0] = 1.0
        m = dict(shared)
        m["xT"] = xT.astype(sd) if MM_MODE == "bf16" else xT
        if MM_MODE == "bf16":
            m["xTr"] = xT
        m["bsr"] = bsr
        m["bsc"] = bsc
        in_maps.append(m)

    res = run_bass_kernel_spmd(nc, in_maps, list(range(NCORES)))
    out = np.empty((T, D), np.float32)
    for cix in range(NCORES):
        out[cix * TC:(cix + 1) * TC, :] = res.results[cix]["outT"].T
    return out.reshape(B, S, D)



# revision 5
# speedup vs baseline: 1.2483x; 1.2483x over previous
"""Trainium2 Bass kernel for nn_MistralMoLoraLayer (MoE-routed LoRA FFN).

Strategy: data-parallel over tokens (8 cores x 256 tokens), base FFN weights
replicated in bf16, all-expert LoRA replicated (resident in SBUF, bf16).

No collectives: the per-(batch,slot) softmax over the sequence axis needs
global denominators, so each core redundantly computes the router (logits +
top-2 + exp) for its WHOLE batch (1024 tokens, +6 MB DMA) and sums the
denominators locally. The batch tokens are rolled per-core on the host so
each core's own 256 tokens land in positions 0:256 -> the program is
SPMD-identical across cores with zero cross-core dependencies (an AllReduce
measured ~0.8 ms/call of sync overhead on hw).

Per-core math (all tiles [h/er/d partitions, tokens free]):
  router: logits = xB @ gate_w.T for the 1024 batch tokens; top-2
          (value,index) per token; exp; local denominator sum; weights
          w_j = exp_j / denom[slot j] for the local 256 tokens
  A-proj: UA/GA [E*R=128, t] = stacked up_A/gate_A @ x.T
  slot-mask trick: Ut_j = UA * M_j where M_j[e*R+r, t] = (sel_j(t)==e);
          lo_up_j[h,t] = (stacked up_B) @ Ut_j  == up_B[sel_j(t)] @ u_sel
  h_j = silu(U + lo_up_j) * (G + lo_gate_j); ch_j = c_j * h_j
  mixed = ch_0 + ch_1
  v_j[er,t] = (stacked down_A) @ ch_j  (accumulated over h), masked by M_j
  outT[d,t] = w_down-chain @ mixed + (stacked down_B) @ v_0 + ... @ v_1
"""

import numpy as np

# problem constants (hardcoded; kernel.py must be self-contained)
B, S, D, H, E, R, TOPK = 2, 1024, 2048, 5632, 8, 16, 2
ALPHA = 2.0
T = B * S
NCORES = 8
TC = T // NCORES           # 256 tokens per core
SB = S                     # batch tokens seen by the router (1024)
KT = D // 128              # 16 k-tiles over D
HT = H // 128              # 44 h-tiles
DT = D // 128              # 16 d-tiles
ER = E * R                 # 128
NBT = SB // 128            # 8 router token-tiles per batch

SKIP_AR = False            # kept for test.py compat; no collective anymore

_cache = {}


def _build():
    import concourse.bacc as bacc
    import concourse.bass as bass
    import concourse.mybir as mybir
    import concourse.tile as tile
    from concourse.masks import make_identity

    f32 = mybir.dt.float32
    bf16 = mybir.dt.bfloat16
    AL = mybir.AluOpType
    AF = mybir.ActivationFunctionType

    nc = bacc.Bacc("TRN2", target_bir_lowering=False, debug=False,
                   num_devices=NCORES)

    # ---- DRAM I/O ----
    d_xT = nc.dram_tensor("xT", [D, TC], bf16, kind="ExternalInput").ap()
    d_xB = nc.dram_tensor("xB", [D, SB], f32, kind="ExternalInput").ap()
    d_gw = nc.dram_tensor("gw", [128, KT * E], f32, kind="ExternalInput").ap()
    d_wu = nc.dram_tensor("wu", [HT, 128, KT * 128], bf16,
                          kind="ExternalInput").ap()
    d_wg = nc.dram_tensor("wg", [HT, 128, KT * 128], bf16,
                          kind="ExternalInput").ap()
    d_wd = nc.dram_tensor("wd", [DT, 128, HT * 128], bf16,
                          kind="ExternalInput").ap()
    d_A = nc.dram_tensor("Ah", [128, KT * 2 * ER], bf16,
                         kind="ExternalInput").ap()
    d_uB = nc.dram_tensor("uB", [128, HT * 128], bf16,
                          kind="ExternalInput").ap()
    d_gB = nc.dram_tensor("gB", [128, HT * 128], bf16,
                          kind="ExternalInput").ap()
    d_dA = nc.dram_tensor("dA", [128, HT * 128], bf16,
                          kind="ExternalInput").ap()
    d_dB = nc.dram_tensor("dB", [128, D], bf16, kind="ExternalInput").ap()
    d_eid = nc.dram_tensor("eid", [128, 1], f32, kind="ExternalInput").ap()
    d_i8m = nc.dram_tensor("i8m", [128, E], f32, kind="ExternalInput").ap()
    d_sel2 = nc.dram_tensor("sel2", [2, 256], f32, kind="ExternalInput").ap()
    d_out = nc.dram_tensor("outT", [D, TC], f32, kind="ExternalOutput").ap()

    with tile.TileContext(nc) as tc:
        import contextlib
        ctx = contextlib.ExitStack()
        with ctx:
            cpool = ctx.enter_context(tc.tile_pool(name="const", bufs=1))
            wpool = ctx.enter_context(tc.tile_pool(name="wstream", bufs=2))
            spool = ctx.enter_context(tc.tile_pool(name="work", bufs=2))
            pspool = ctx.enter_context(
                tc.tile_pool(name="ps", bufs=1, space="PSUM"))

            # ---- constants / resident tiles ----
            xT_sb = cpool.tile([128, KT * TC], bf16, name="xT_sb")
            for k in range(KT):
                nc.sync.dma_start(out=xT_sb[:, k * TC:(k + 1) * TC],
                                  in_=d_xT[k * 128:(k + 1) * 128, :])
            A_sb = cpool.tile([128, KT * 2 * ER], bf16, name="A_sb")
            nc.sync.dma_start(out=A_sb[:], in_=d_A[:])
            uB_sb = cpool.tile([128, HT * 128], bf16, name="uB_sb")
            nc.sync.dma_start(out=uB_sb[:], in_=d_uB[:])
            gB_sb = cpool.tile([128, HT * 128], bf16, name="gB_sb")
            nc.sync.dma_start(out=gB_sb[:], in_=d_gB[:])
            dA_sb = cpool.tile([128, HT * 128], bf16, name="dA_sb")
            nc.sync.dma_start(out=dA_sb[:], in_=d_dA[:])
            dB_sb = cpool.tile([128, D], bf16, name="dB_sb")
            nc.sync.dma_start(out=dB_sb[:], in_=d_dB[:])
            gw_sb = cpool.tile([128, KT * E], f32, name="gw_sb")
            nc.sync.dma_start(out=gw_sb[:], in_=d_gw[:])
            eid_sb = cpool.tile([128, 1], f32, name="eid_sb")
            nc.sync.dma_start(out=eid_sb[:], in_=d_eid[:])
            i8m_sb = cpool.tile([128, E], f32, name="i8m_sb")
            nc.sync.dma_start(out=i8m_sb[:], in_=d_i8m[:])
            sel2_sb = cpool.tile([2, 256], f32, name="sel2_sb")
            nc.sync.dma_start(out=sel2_sb[:], in_=d_sel2[:])

            ident = cpool.tile([128, 128], f32, name="ident")
            make_identity(nc, ident)
            ones_col = cpool.tile([128, 1], f32, name="ones_col")
            nc.vector.memset(ones_col, 1.0)

            mixed = cpool.tile([128, HT * TC], bf16, name="mixed")
            ev_rows = cpool.tile([2, TC], f32, name="ev_rows")
            s_rows = cpool.tile([2, TC], f32, name="s_rows")
            crows = cpool.tile([2, TC], f32, name="crows")
            cb = cpool.tile([128, 2 * TC], bf16, name="cb")
            Mj = cpool.tile([128, 2 * TC], bf16, name="Mj")
            UA = cpool.tile([128, TC], bf16, name="UA")
            GA = cpool.tile([128, TC], bf16, name="GA")
            Ut = cpool.tile([128, 2 * TC], bf16, name="Ut")
            Gt = cpool.tile([128, 2 * TC], bf16, name="Gt")
            vt = cpool.tile([128, 2 * TC], bf16, name="vt")
            dacc = cpool.tile([128, 2], f32, name="dacc")

            # ---- phase 1: router over the whole batch (1024 tokens) ----
            # logits token-major: psL pairs [128,512] hold two token-tiles'
            # [128,8] logit blocks at cols 0 and 256. 4 pairs = 4 PSUM banks
            # live during the k-chain; xB streamed k-major once.
            psLp = [pspool.tile([128, 2 * TC], f32, tag="psLO", bufs=4,
                                name=f"psLp{p}") for p in range(4)]
            for k in range(KT):
                xb_t = wpool.tile([128, SB], f32, tag="xb", bufs=3)
                nc.sync.dma_start(out=xb_t[:],
                                  in_=d_xB[k * 128:(k + 1) * 128, :])
                for p in range(4):
                    for h in range(2):
                        # ONE start per bank: a second start=True in the same
                        # bank clears the whole bank's has_written bits and
                        # the sibling region's accumulation loses its k=0
                        # term. The h=1 region's first write lands on cleared
                        # has_written and correctly overwrites.
                        nc.tensor.matmul(
                            psLp[p][:, h * TC:h * TC + E],
                            xb_t[:, (2 * p + h) * 128:(2 * p + h + 1) * 128],
                            gw_sb[:, k * E:(k + 1) * E],
                            start=(k == 0 and h == 0), stop=(k == KT - 1),
                            skip_group_check=(h == 1))

            for tt in range(NBT):
                psL = psLp[tt // 2][:, (tt % 2) * TC:(tt % 2) * TC + E]
                L = spool.tile([128, E], f32, tag="L", bufs=3)
                nc.vector.tensor_copy(L[:], psL)
                mx1 = spool.tile([128, 1], f32, tag="mx1")
                nc.vector.tensor_reduce(mx1[:], L[:], mybir.AxisListType.X,
                                        AL.max)
                msk = spool.tile([128, E], f32, tag="msk")
                nc.vector.tensor_scalar(msk[:], L[:], mx1[:], None,
                                        AL.is_equal)
                evals = spool.tile([128, 2], f32, tag="evals", bufs=3)
                nc.scalar.activation(evals[:, 0:1], mx1[:], AF.Exp)
                # mask out slot-0 winner, find second max
                big = spool.tile([128, E], f32, tag="big")
                nc.vector.tensor_scalar(big[:], msk[:], 1e30, None, AL.mult)
                L2 = spool.tile([128, E], f32, tag="L2")
                nc.vector.tensor_tensor(L2[:], L[:], big[:], AL.subtract)
                mx2 = spool.tile([128, 1], f32, tag="mx2")
                nc.vector.tensor_reduce(mx2[:], L2[:], mybir.AxisListType.X,
                                        AL.max)
                nc.scalar.activation(evals[:, 1:2], mx2[:], AF.Exp)
                # denominator accumulation (all 8 tiles)
                if tt == 0:
                    nc.vector.tensor_copy(dacc[:], evals[:])
                else:
                    nc.vector.tensor_tensor(dacc[:], dacc[:], evals[:],
                                            AL.add)
                if tt < 2:
                    # local tokens: need expert ids + value rows
                    svals = spool.tile([128, 2], f32, tag="svals")
                    mi = spool.tile([128, E], f32, tag="mi")
                    nc.vector.tensor_tensor(mi[:], msk[:], i8m_sb[:], AL.mult)
                    nc.vector.tensor_reduce(svals[:, 0:1], mi[:],
                                            mybir.AxisListType.X, AL.max)
                    msk2 = spool.tile([128, E], f32, tag="msk2")
                    nc.vector.tensor_scalar(msk2[:], L2[:], mx2[:], None,
                                            AL.is_equal)
                    mi2 = spool.tile([128, E], f32, tag="mi2")
                    nc.vector.tensor_tensor(mi2[:], msk2[:], i8m_sb[:],
                                            AL.mult)
                    nc.vector.tensor_reduce(svals[:, 1:2], mi2[:],
                                            mybir.AxisListType.X, AL.max)
                    # transpose evals/svals -> rows
                    psT = pspool.tile([2, 128], f32, tag="ps_small",
                                      name="psT")
                    nc.tensor.transpose(psT[:], evals[:], ident[:])
                    nc.vector.tensor_copy(
                        ev_rows[:, tt * 128:(tt + 1) * 128], psT[:])
                    psT2 = pspool.tile([2, 128], f32, tag="ps_small",
                                       name="psT2")
                    nc.tensor.transpose(psT2[:], svals[:], ident[:])
                    nc.vector.tensor_copy(
                        s_rows[:, tt * 128:(tt + 1) * 128], psT2[:])

            # denominators: [2,1] = dacc.T @ ones, reciprocal
            psDC = pspool.tile([2, 1], f32, tag="ps_small", name="psDC")
            nc.tensor.matmul(psDC[:], dacc[:], ones_col[:],
                             start=True, stop=True)
            rcp = cpool.tile([2, 1], f32, name="rcp")
            nc.vector.reciprocal(rcp[:], psDC[:])
            # normalized routing weights as rows [2, TC]
            nc.vector.tensor_scalar(crows[:], ev_rows[:], rcp[:], None,
                                    AL.mult)

            # broadcast slot rows along partitions via K=2 matmul with a
            # row-selector constant (sel2[:, j*128:(j+1)*128] has row j = 1)
            for j in range(2):
                psB = pspool.tile([128, TC], f32, tag="ps_small",
                                  name="psB")
                nc.tensor.matmul(psB[:], sel2_sb[:, j * 128:(j + 1) * 128],
                                 crows[:], start=True, stop=True)
                nc.vector.tensor_copy(cb[:, j * TC:(j + 1) * TC], psB[:])
                psM = pspool.tile([128, TC], f32, tag="ps_small",
                                  name="psM")
                nc.tensor.matmul(psM[:], sel2_sb[:, j * 128:(j + 1) * 128],
                                 s_rows[:], start=True, stop=True)
                nc.vector.tensor_scalar(Mj[:, j * TC:(j + 1) * TC], psM[:],
                                        eid_sb[:], None, AL.is_equal)

            # ---- phase 3: stacked A-projections ----
            psUA = pspool.tile([128, TC], f32, tag="psUG", bufs=2,
                               name="psUA")
            for k in range(KT):
                nc.tensor.matmul(psUA[:],
                                 A_sb[:, k * 2 * ER: k * 2 * ER + ER],
                                 xT_sb[:, k * TC:(k + 1) * TC],
                                 start=(k == 0), stop=(k == KT - 1))
            nc.vector.tensor_copy(UA[:], psUA[:])
            psGA = pspool.tile([128, TC], f32, tag="psUG", bufs=2,
                               name="psGA")
            for k in range(KT):
                nc.tensor.matmul(psGA[:],
                                 A_sb[:, k * 2 * ER + ER:(k + 1) * 2 * ER],
                                 xT_sb[:, k * TC:(k + 1) * TC],
                                 start=(k == 0), stop=(k == KT - 1))
            nc.vector.tensor_copy(GA[:], psGA[:])
            for j in range(2):
                nc.vector.tensor_tensor(Ut[:, j * TC:(j + 1) * TC], UA[:],
                                        Mj[:, j * TC:(j + 1) * TC], AL.mult)
                nc.vector.tensor_tensor(Gt[:, j * TC:(j + 1) * TC], GA[:],
                                        Mj[:, j * TC:(j + 1) * TC], AL.mult)

            # ---- phases 2+5+6: h-tile loop ----
            psV = pspool.tile([128, 2 * TC], f32, tag="psV", name="psV")
            KH = KT // 2 * 128          # half of the k columns (1024)
            pend_v = None               # delayed psV matmul (i, ch_pair)

            for i in range(HT):
                wu_h = []
                wg_h = []
                for hf in range(2):
                    wu_t = wpool.tile([128, KH], bf16, tag="wu", bufs=4)
                    nc.sync.dma_start(
                        out=wu_t[:],
                        in_=d_wu[i][:, hf * KH:(hf + 1) * KH])
                    wu_h.append(wu_t)
                    wg_t = wpool.tile([128, KH], bf16, tag="wg", bufs=4)
                    nc.sync.dma_start(
                        out=wg_t[:],
                        in_=d_wg[i][:, hf * KH:(hf + 1) * KH])
                    wg_h.append(wg_t)

                psUG = pspool.tile([128, 2 * TC], f32, tag="psUG", bufs=2,
                                   name="psUG")
                for k in range(KT):
                    w = wu_h[k // 8][:, (k % 8) * 128:(k % 8 + 1) * 128]
                    nc.tensor.matmul(psUG[:, 0:TC], w,
                                     xT_sb[:, k * TC:(k + 1) * TC],
                                     start=(k == 0), stop=(k == KT - 1))
                for k in range(KT):
                    w = wg_h[k // 8][:, (k % 8) * 128:(k % 8 + 1) * 128]
                    nc.tensor.matmul(psUG[:, TC:2 * TC], w,
                                     xT_sb[:, k * TC:(k + 1) * TC],
                                     start=(k == 0), stop=(k == KT - 1))
                if pend_v is not None:
                    pv_i, pv_ch = pend_v
                    nc.tensor.matmul(psV[:],
                                     dA_sb[:, pv_i * 128:(pv_i + 1) * 128],
                                     pv_ch[:],
                                     start=(i == 1), stop=False,
                                     skip_group_check=True)

                # stage [U | G] to SBUF once (DVE may read only ONE input
                # from PSUM per op, so the adds below use UG_sb + psLO)
                UG_sb = spool.tile([128, 2 * TC], bf16, tag="UG")
                nc.scalar.copy(UG_sb[:], psUG[:])

                # both slots' c*h in ONE tile so the down_A contraction is a
                # single [128,512] matmul per h-tile
                ch_pair = spool.tile([128, 2 * TC], bf16, tag="chp", bufs=3)
                for j in range(2):
                    # psLO_j = [lo_up_j | lo_gate_j], one PSUM bank
                    psLO = pspool.tile([128, 2 * TC], f32, tag="psLO",
                                       bufs=4, name="psLO")
                    nc.tensor.matmul(psLO[:, 0:TC],
                                     uB_sb[:, i * 128:(i + 1) * 128],
                                     Ut[:, j * TC:(j + 1) * TC],
                                     start=True, stop=True)
                    nc.tensor.matmul(psLO[:, TC:2 * TC],
                                     gB_sb[:, i * 128:(i + 1) * 128],
                                     Gt[:, j * TC:(j + 1) * TC],
                                     start=True, stop=True)
                    # [U+lo_u | G+lo_g] in one 512-wide add
                    tusg = spool.tile([128, 2 * TC], bf16, tag="tusg")
                    nc.vector.tensor_tensor(tusg[:], UG_sb[:], psLO[:],
                                            AL.add)
                    su = spool.tile([128, TC], bf16, tag="su")
                    nc.scalar.activation(su[:], tusg[:, 0:TC], AF.Silu)
                    hh = spool.tile([128, TC], bf16, tag="hh")
                    nc.vector.tensor_tensor(hh[:], su[:], tusg[:, TC:2 * TC],
                                            AL.mult)
                    nc.vector.tensor_tensor(ch_pair[:, j * TC:(j + 1) * TC],
                                            hh[:],
                                            cb[:, j * TC:(j + 1) * TC],
                                            AL.mult)
                nc.gpsimd.tensor_tensor(mixed[:, i * TC:(i + 1) * TC],
                                        ch_pair[:, 0:TC],
                                        ch_pair[:, TC:2 * TC], AL.add)
                pend_v = (i, ch_pair)

            pv_i, pv_ch = pend_v
            nc.tensor.matmul(psV[:], dA_sb[:, pv_i * 128:(pv_i + 1) * 128],
                             pv_ch[:], start=False, stop=True,
                             skip_group_check=True)
            # masked v (one 512-wide mult)
            nc.vector.tensor_tensor(vt[:], psV[:], Mj[:], AL.mult)

            # ---- phase 7: down GEMM + LoRA-down ----
            for di in range(DT):
                wd_q = []
                for q in range(4):
                    t = wpool.tile([128, 11 * 128], bf16, tag="wd", bufs=8)
                    nc.sync.dma_start(
                        out=t[:],
                        in_=d_wd[di][:, q * 11 * 128:(q + 1) * 11 * 128])
                    wd_q.append(t)
                psO = pspool.tile([128, TC], f32, tag="psUG", bufs=2,
                                  name="psO")
                for hk in range(HT):
                    w = wd_q[hk // 11][:, (hk % 11) * 128:(hk % 11 + 1) * 128]
                    nc.tensor.matmul(psO[:], w,
                                     mixed[:, hk * TC:(hk + 1) * TC],
                                     start=(hk == 0), stop=False,
                                     skip_group_check=True)
                nc.tensor.matmul(psO[:], dB_sb[:, di * 128:(di + 1) * 128],
                                 vt[:, 0:TC], start=False, stop=False,
                                 skip_group_check=True)
                nc.tensor.matmul(psO[:], dB_sb[:, di * 128:(di + 1) * 128],
                                 vt[:, TC:2 * TC], start=False, stop=True,
                                 skip_group_check=True)
                o_sb = spool.tile([128, TC], f32, tag="o_sb")
                nc.scalar.copy(o_sb[:], psO[:])
                nc.sync.dma_start(out=d_out[di * 128:(di + 1) * 128, :],
                                  in_=o_sb[:])

    nc.compile()
    return nc


def _prep_shared(inputs):
    """Host-side layout prep of weight tensors (shared across cores)."""
    import ml_dtypes
    bf16 = np.dtype(ml_dtypes.bfloat16)
    f32 = np.float32

    def c(a, dt):
        return np.ascontiguousarray(a.astype(dt, copy=False))

    w_up, w_gate, w_down = inputs["w_up"], inputs["w_gate"], inputs["w_down"]
    wu = c(w_up.reshape(HT, 128, KT, 128).transpose(0, 3, 2, 1)
           .reshape(HT, 128, KT * 128), bf16)
    wg = c(w_gate.reshape(HT, 128, KT, 128).transpose(0, 3, 2, 1)
           .reshape(HT, 128, KT * 128), bf16)
    wd = c(w_down.reshape(DT, 128, HT, 128).transpose(0, 3, 2, 1)
           .reshape(DT, 128, HT * 128), bf16)

    A_stack = np.concatenate([
        inputs["up_A"].reshape(ER, D),
        inputs["gate_A"].reshape(ER, D)], axis=0)          # [2*ER, D]
    # Ah[p, k*2ER + m] = A_stack[m, k*128+p]
    Ah = c(A_stack.reshape(2 * ER, KT, 128).transpose(2, 1, 0)
           .reshape(128, KT * 2 * ER), bf16)

    # resident B tensors: [er=128 partitions, HT*128 cols] (tile-major cols)
    up_B_all = (inputs["up_B"].transpose(0, 2, 1).reshape(ER, H)
                * ALPHA).astype(f32)
    gate_B_all = (inputs["gate_B"].transpose(0, 2, 1).reshape(ER, H)
                  * ALPHA).astype(f32)
    uB = c(up_B_all, bf16)                                 # [128, H]
    gB = c(gate_B_all, bf16)
    down_A_all = inputs["down_A"].reshape(ER, H).astype(f32)
    # dA cols tile-major with h-within-tile: dA[p=h%128? no:
    # dA_sb slice [:, i*128:(i+1)*128] must be lhsT [h=128, er.. wait
    # lhsT for psV: [h-part, er-free]: dA_res[p, i*128+er] = down_A[er, i*128+p]
    dA = c(down_A_all.T.reshape(HT, 128, ER).transpose(1, 0, 2)
           .reshape(128, HT * ER), bf16)
    down_B_all = (inputs["down_B"].transpose(0, 2, 1).reshape(ER, D)
                  * ALPHA).astype(f32)
    dB = c(down_B_all, bf16)

    gate_wT = inputs["gate_w"].T.astype(f32)               # [D, E]
    gw = c(gate_wT.reshape(KT, 128, E).transpose(1, 0, 2)
           .reshape(128, KT * E), f32)

    eid = (8.0 - (np.arange(128) // R)).astype(f32).reshape(128, 1)
    i8m = np.tile((8.0 - np.arange(E)).astype(f32), (128, 1))
    sel2 = np.zeros((2, 256), f32)
    sel2[0, 0:128] = 1.0
    sel2[1, 128:256] = 1.0

    return dict(wu=wu, wg=wg, wd=wd, Ah=Ah, uB=uB, gB=gB, dA=dA, dB=dB,
                gw=gw, eid=eid, i8m=i8m, sel2=sel2)


def _in_maps(inputs):
    """Build per-core input maps (shared weights + per-core x slices)."""
    import ml_dtypes
    bf16 = np.dtype(ml_dtypes.bfloat16)
    shared = _prep_shared(inputs)
    x = np.asarray(inputs["x"]).astype(np.float32)
    xt = x.reshape(T, D)

    maps = []
    for cix in range(NCORES):
        b = (cix * TC) // S
        o = (cix * TC) % S                                 # offset in batch
        xb = xt[b * S:(b + 1) * S]                         # [S, D] batch
        rolled = np.concatenate([xb[o:], xb[:o]], axis=0)  # local 256 first
        m = dict(shared)
        m["xB"] = np.ascontiguousarray(rolled.T)           # [D, S] f32
        m["xT"] = np.ascontiguousarray(rolled[0:TC].T.astype(bf16))
        maps.append(m)
    return maps


def kernel(**inputs):
    from concourse.bass_utils import run_bass_kernel_spmd

    inputs = {k: np.asarray(v) for k, v in inputs.items()}
    if "nc" not in _cache:
        _cache["nc"] = _build()
    nc = _cache["nc"]

    in_maps = _in_maps(inputs)
    res = run_bass_kernel_spmd(nc, in_maps, list(range(NCORES)))
    out = np.empty((T, D), np.float32)
    for cix in range(NCORES):
        out[cix * TC:(cix + 1) * TC, :] = res.results[cix]["outT"].T
    return out.reshape(B, S, D)


# revision 14
# speedup vs baseline: 1.3407x; 1.0740x over previous
"""Trainium2 Bass kernel for nn_MistralMoLoraLayer (MoE-routed LoRA FFN).

Strategy: data-parallel over tokens (8 cores x 256 tokens), base FFN weights
replicated in bf16, all-expert LoRA replicated (resident in SBUF, bf16).

No collectives: the per-(batch,slot) softmax over the sequence axis needs
global denominators, so each core redundantly computes the router (logits +
top-2 + exp) for its WHOLE batch (1024 tokens) and sums the denominators
locally (an AllReduce measured ~0.8 ms/call of sync overhead on hw). The
batch tokens are rolled per-core on the host so each core's own 256 tokens
land in positions 0:256 -> the program is SPMD-identical across cores. The
local 256 tokens' logits are computed in f32 (they pick experts + weights);
the other 768 feed only the denominator sum, where bf16 rounding averages
out, so they stream as bf16 to cut front-of-kernel DMA pressure.

Schedule: the first STASH_P h-tiles' base GEMMs run BEFORE the router's
matmuls in PE program order (staging U|G to SBUF) so the PE stays busy
while the router token stream is in flight; their LoRA + elementwise are
deferred and interleaved one-per-tile into the main loop. SBUF-only
multiplies run on the otherwise-idle GpSimd engine to keep DVE under the
PE pace.

Per-core math (all tiles [h/er/d partitions, tokens free]):
  router: logits = xB @ gate_w.T for the 1024 batch tokens; top-2
          (value,index) per token; exp; local denominator sum; weights
          w_j = exp_j / denom[slot j] for the local 256 tokens
  A-proj: UA/GA [E*R=128, t] = stacked up_A/gate_A @ x.T
  slot-mask trick: Ut_j = UA * M_j where M_j[e*R+r, t] = (sel_j(t)==e);
          lo_up_j[h,t] = (stacked up_B) @ Ut_j  == up_B[sel_j(t)] @ u_sel
  h_j = silu(U + lo_up_j) * (G + lo_gate_j); ch_j = c_j * h_j
  mixed = ch_0 + ch_1
  v_j[er,t] = (stacked down_A) @ ch_j  (accumulated over h), masked by M_j
  outT[d,t] = w_down-chain @ mixed + (stacked down_B) @ v_0 + ... @ v_1
"""

import numpy as np

# problem constants (hardcoded; kernel.py must be self-contained)
B, S, D, H, E, R, TOPK = 2, 1024, 2048, 5632, 8, 16, 2
ALPHA = 2.0
T = B * S
NCORES = 8
TC = T // NCORES           # 256 tokens per core
SB = S                     # batch tokens seen by the router (1024)
SR = SB - TC               # non-local batch tokens (768)
KT = D // 128              # 16 k-tiles over D
HT = H // 128              # 44 h-tiles
DT = D // 128              # 16 d-tiles
ER = E * R                 # 128
STASH_P = 16               # h-tiles whose base GEMM runs before the router

SKIP_AR = False            # kept for test.py compat; no collective anymore

_cache = {}


def _build():
    import concourse.bacc as bacc
    import concourse.bass as bass
    import concourse.mybir as mybir
    import concourse.tile as tile
    from concourse.masks import make_identity

    f32 = mybir.dt.float32
    bf16 = mybir.dt.bfloat16
    AL = mybir.AluOpType
    AF = mybir.ActivationFunctionType

    nc = bacc.Bacc("TRN2", target_bir_lowering=False, debug=False,
                   num_devices=NCORES)

    # ---- DRAM I/O ----
    d_xT = nc.dram_tensor("xT", [128, KT * TC], bf16,
                          kind="ExternalInput").ap()
    d_xL = nc.dram_tensor("xL", [128, KT * TC], f32,
                          kind="ExternalInput").ap()
    d_xR = nc.dram_tensor("xR", [128, KT * SR], bf16,
                          kind="ExternalInput").ap()
    d_gw = nc.dram_tensor("gw", [128, KT * E], f32, kind="ExternalInput").ap()
    d_gwb = nc.dram_tensor("gwb", [128, KT * E], bf16,
                           kind="ExternalInput").ap()
    d_wug = nc.dram_tensor("wug", [HT, 128, 2 * KT * 128], bf16,
                           kind="ExternalInput").ap()
    d_wd = nc.dram_tensor("wd", [DT, 128, HT * 128], bf16,
                          kind="ExternalInput").ap()
    d_A = nc.dram_tensor("Ah", [128, KT * 2 * ER], bf16,
                         kind="ExternalInput").ap()
    d_uB = nc.dram_tensor("uB", [128, HT * 128], bf16,
                          kind="ExternalInput").ap()
    d_gB = nc.dram_tensor("gB", [128, HT * 128], bf16,
                          kind="ExternalInput").ap()
    d_dA = nc.dram_tensor("dA", [128, HT * 128], bf16,
                          kind="ExternalInput").ap()
    d_dB = nc.dram_tensor("dB", [128, D], bf16, kind="ExternalInput").ap()
    d_eid = nc.dram_tensor("eid", [128, 1], f32, kind="ExternalInput").ap()
    d_i8m = nc.dram_tensor("i8m", [128, E], f32, kind="ExternalInput").ap()
    d_sel2 = nc.dram_tensor("sel2", [2, 256], f32, kind="ExternalInput").ap()
    d_out = nc.dram_tensor("outT", [D, TC], f32, kind="ExternalOutput").ap()

    with tile.TileContext(nc) as tc:
        import contextlib
        ctx = contextlib.ExitStack()
        with ctx:
            cpool = ctx.enter_context(tc.tile_pool(name="const", bufs=1))
            wpool = ctx.enter_context(tc.tile_pool(name="wstream", bufs=2))
            spool = ctx.enter_context(tc.tile_pool(name="work", bufs=2))
            pspool = ctx.enter_context(
                tc.tile_pool(name="ps", bufs=1, space="PSUM"))

            # ---- xT first: everything needs it; other consts interleave
            # into pass A so the first base chain starts ASAP ----
            xT_sb = cpool.tile([128, KT * TC], bf16, name="xT_sb")
            XH = KT * TC // 2
            nc.sync.dma_start(out=xT_sb[:, 0:XH], in_=d_xT[:, 0:XH])
            nc.sync.dma_start(out=xT_sb[:, XH:], in_=d_xT[:, XH:])
            gw_sb = cpool.tile([128, KT * E], f32, name="gw_sb")
            gwb_sb = cpool.tile([128, KT * E], bf16, name="gwb_sb")
            eid_sb = cpool.tile([128, 1], f32, name="eid_sb")
            i8m_sb = cpool.tile([128, E], f32, name="i8m_sb")
            sel2_sb = cpool.tile([2, 256], f32, name="sel2_sb")
            A_sb = cpool.tile([128, KT * 2 * ER], bf16, name="A_sb")

            ident = cpool.tile([128, 128], f32, name="ident")
            make_identity(nc, ident)
            ones_col = cpool.tile([128, 1], f32, name="ones_col")
            nc.vector.memset(ones_col, 1.0)

            mixed = cpool.tile([128, HT * TC], bf16, name="mixed")
            ev_rows = cpool.tile([2, TC], f32, name="ev_rows")
            s_rows = cpool.tile([2, TC], f32, name="s_rows")
            crows = cpool.tile([2, TC], f32, name="crows")
            cb = cpool.tile([128, 2 * TC], bf16, name="cb")
            Mj = cpool.tile([128, 2 * TC], bf16, name="Mj")
            UA = cpool.tile([128, TC], bf16, name="UA")
            GA = cpool.tile([128, TC], bf16, name="GA")
            Ut = cpool.tile([128, 2 * TC], bf16, name="Ut")
            Gt = cpool.tile([128, 2 * TC], bf16, name="Gt")
            vt = cpool.tile([128, 2 * TC], bf16, name="vt")
            dacc = cpool.tile([128, 2], f32, name="dacc")
            UGstash = cpool.tile([128, STASH_P * 2 * TC], bf16,
                                 name="UGstash")

            WH = KT * 128             # 2048 cols per wug half (up | gate)

            def load_wug(i):
                t = wpool.tile([128, 2 * KT * 128], bf16, tag="wug", bufs=3)
                nc.sync.dma_start(out=t[:, 0:WH], in_=d_wug[i][:, 0:WH])
                nc.sync.dma_start(out=t[:, WH:], in_=d_wug[i][:, WH:])
                return t

            def base_chain(i, wugh):
                psUG = pspool.tile([128, 2 * TC], f32, tag="psUG", bufs=2,
                                   name="psUG")
                for k in range(KT):
                    nc.tensor.matmul(psUG[:, 0:TC],
                                     wugh[:, k * 128:(k + 1) * 128],
                                     xT_sb[:, k * TC:(k + 1) * TC],
                                     start=(k == 0), stop=(k == KT - 1))
                for k in range(KT):
                    nc.tensor.matmul(psUG[:, TC:2 * TC],
                                     wugh[:, (KT + k) * 128:(KT + k + 1) * 128],
                                     xT_sb[:, k * TC:(k + 1) * TC],
                                     start=(k == 0), stop=(k == KT - 1))
                return psUG

            # ---- pass A: base GEMMs for the first STASH_P tiles, with the
            # router token stream's DMAs interleaved into the queue (one
            # k-slice per pass-A tile, so wu/wg and xL/xR share bandwidth
            # and the first base chain isn't queued behind the full 5.6 MB
            # router stream) ----
            xl_sb = cpool.tile([128, KT * TC], f32, name="xl_sb")
            xr_sb = cpool.tile([128, KT * SR], bf16, name="xr_sb")
            for i in range(STASH_P):
                wugh = load_wug(i)
                if i == 1:
                    nc.sync.dma_start(out=gw_sb[:], in_=d_gw[:])
                    nc.sync.dma_start(out=gwb_sb[:], in_=d_gwb[:])
                elif i == 3:
                    nc.sync.dma_start(out=xl_sb[:, 0:XH], in_=d_xL[:, 0:XH])
                elif i == 5:
                    nc.sync.dma_start(out=xl_sb[:, XH:], in_=d_xL[:, XH:])
                elif i == 7:
                    nc.sync.dma_start(out=xr_sb[:, 0:KT * SR // 2],
                                      in_=d_xR[:, 0:KT * SR // 2])
                elif i == 9:
                    nc.sync.dma_start(out=xr_sb[:, KT * SR // 2:],
                                      in_=d_xR[:, KT * SR // 2:])
                elif i == 11:
                    nc.sync.dma_start(out=A_sb[:], in_=d_A[:])
                elif i == 13:
                    nc.sync.dma_start(out=eid_sb[:], in_=d_eid[:])
                    nc.sync.dma_start(out=i8m_sb[:], in_=d_i8m[:])
                    nc.sync.dma_start(out=sel2_sb[:], in_=d_sel2[:])
                psUG = base_chain(i, wugh)
                nc.scalar.copy(
                    UGstash[:, i * 2 * TC:(i + 1) * 2 * TC], psUG[:])


            # ---- router matmuls (xL/xR fully streamed by now) ----
            # psL pairs [128,512]: two token-tiles' [128,8] logit blocks at
            # cols 0 and 256. ONE start per bank (a second start=True would
            # clear the whole bank's has_written and break the sibling
            # region's accumulation); the h=1 region's first write lands on
            # cleared has_written and correctly overwrites.
            psLp = [pspool.tile([128, 2 * TC], f32, tag="psLO", bufs=4,
                                name=f"psLp{p}") for p in range(4)]
            for k in range(KT):
                for h in range(2):
                    nc.tensor.matmul(
                        psLp[0][:, h * TC:h * TC + E],
                        xl_sb[:, k * TC + h * 128:k * TC + (h + 1) * 128],
                        gw_sb[:, k * E:(k + 1) * E],
                        start=(k == 0 and h == 0), stop=(k == KT - 1),
                        skip_group_check=(h == 1))
                for p in range(1, 4):
                    for h in range(2):
                        nc.tensor.matmul(
                            psLp[p][:, h * TC:h * TC + E],
                            xr_sb[:, k * SR + (2 * p + h - 2) * 128:
                                  k * SR + (2 * p + h - 1) * 128],
                            gwb_sb[:, k * E:(k + 1) * E],
                            start=(k == 0 and h == 0), stop=(k == KT - 1),
                            skip_group_check=(h == 1))

            for tt in range(2 * 4):
                psL = psLp[tt // 2][:, (tt % 2) * TC:(tt % 2) * TC + E]
                L = spool.tile([128, E], f32, tag="L", bufs=3)
                nc.vector.tensor_copy(L[:], psL)
                mx1 = spool.tile([128, 1], f32, tag="mx1")
                nc.vector.tensor_reduce(mx1[:], L[:], mybir.AxisListType.X,
                                        AL.max)
                msk = spool.tile([128, E], f32, tag="msk")
                nc.vector.tensor_scalar(msk[:], L[:], mx1[:], None,
                                        AL.is_equal)
                evals = spool.tile([128, 2], f32, tag="evals", bufs=3)
                nc.scalar.activation(evals[:, 0:1], mx1[:], AF.Exp)
                # mask out slot-0 winner, find second max
                big = spool.tile([128, E], f32, tag="big")
                nc.vector.tensor_scalar(big[:], msk[:], 1e30, None, AL.mult)
                L2 = spool.tile([128, E], f32, tag="L2")
                nc.vector.tensor_tensor(L2[:], L[:], big[:], AL.subtract)
                mx2 = spool.tile([128, 1], f32, tag="mx2")
                nc.vector.tensor_reduce(mx2[:], L2[:], mybir.AxisListType.X,
                                        AL.max)
                nc.scalar.activation(evals[:, 1:2], mx2[:], AF.Exp)
                # denominator accumulation (all 8 tiles)
                if tt == 0:
                    nc.vector.tensor_copy(dacc[:], evals[:])
                else:
                    nc.vector.tensor_tensor(dacc[:], dacc[:], evals[:],
                                            AL.add)
                if tt < 2:
                    # local tokens: need expert ids + value rows
                    svals = spool.tile([128, 2], f32, tag="svals")
                    mi = spool.tile([128, E], f32, tag="mi")
                    nc.vector.tensor_tensor(mi[:], msk[:], i8m_sb[:], AL.mult)
                    nc.vector.tensor_reduce(svals[:, 0:1], mi[:],
                                            mybir.AxisListType.X, AL.max)
                    msk2 = spool.tile([128, E], f32, tag="msk2")
                    nc.vector.tensor_scalar(msk2[:], L2[:], mx2[:], None,
                                            AL.is_equal)
                    mi2 = spool.tile([128, E], f32, tag="mi2")
                    nc.vector.tensor_tensor(mi2[:], msk2[:], i8m_sb[:],
                                            AL.mult)
                    nc.vector.tensor_reduce(svals[:, 1:2], mi2[:],
                                            mybir.AxisListType.X, AL.max)
                    # transpose evals/svals -> rows
                    psT = pspool.tile([2, 128], f32, tag="ps_small",
                                      name="psT")
                    nc.tensor.transpose(psT[:], evals[:], ident[:])
                    nc.vector.tensor_copy(
                        ev_rows[:, tt * 128:(tt + 1) * 128], psT[:])
                    psT2 = pspool.tile([2, 128], f32, tag="ps_small",
                                       name="psT2")
                    nc.tensor.transpose(psT2[:], svals[:], ident[:])
                    nc.vector.tensor_copy(
                        s_rows[:, tt * 128:(tt + 1) * 128], psT2[:])

            # denominators: [2,1] = dacc.T @ ones, reciprocal
            psDC = pspool.tile([2, 1], f32, tag="ps_small", name="psDC")
            nc.tensor.matmul(psDC[:], dacc[:], ones_col[:],
                             start=True, stop=True)
            rcp = cpool.tile([2, 1], f32, name="rcp")
            nc.vector.reciprocal(rcp[:], psDC[:])
            # normalized routing weights as rows [2, TC]
            nc.vector.tensor_scalar(crows[:], ev_rows[:], rcp[:], None,
                                    AL.mult)

            # broadcast slot rows along partitions via K=2 matmul with a
            # row-selector constant (sel2[:, j*128:(j+1)*128] has row j = 1)
            for j in range(2):
                psBr = pspool.tile([128, TC], f32, tag="ps_small",
                                   name="psBr")
                nc.tensor.matmul(psBr[:], sel2_sb[:, j * 128:(j + 1) * 128],
                                 crows[:], start=True, stop=True)
                nc.vector.tensor_copy(cb[:, j * TC:(j + 1) * TC], psBr[:])
                psM = pspool.tile([128, TC], f32, tag="ps_small",
                                  name="psM")
                nc.tensor.matmul(psM[:], sel2_sb[:, j * 128:(j + 1) * 128],
                                 s_rows[:], start=True, stop=True)
                nc.vector.tensor_scalar(Mj[:, j * TC:(j + 1) * TC], psM[:],
                                        eid_sb[:], None, AL.is_equal)

            # ---- A-proj (needs only xT+A; results used post-router) ----
            psUA = pspool.tile([128, TC], f32, tag="psUG", bufs=2,
                               name="psUA")
            for k in range(KT):
                nc.tensor.matmul(psUA[:],
                                 A_sb[:, k * 2 * ER: k * 2 * ER + ER],
                                 xT_sb[:, k * TC:(k + 1) * TC],
                                 start=(k == 0), stop=(k == KT - 1))
            nc.vector.tensor_copy(UA[:], psUA[:])
            psGA = pspool.tile([128, TC], f32, tag="psUG", bufs=2,
                               name="psGA")
            for k in range(KT):
                nc.tensor.matmul(psGA[:],
                                 A_sb[:, k * 2 * ER + ER:(k + 1) * 2 * ER],
                                 xT_sb[:, k * TC:(k + 1) * TC],
                                 start=(k == 0), stop=(k == KT - 1))
            nc.vector.tensor_copy(GA[:], psGA[:])

            for j in range(2):
                nc.vector.tensor_tensor(Ut[:, j * TC:(j + 1) * TC], UA[:],
                                        Mj[:, j * TC:(j + 1) * TC], AL.mult)
                nc.vector.tensor_tensor(Gt[:, j * TC:(j + 1) * TC], GA[:],
                                        Mj[:, j * TC:(j + 1) * TC], AL.mult)

            # resident LoRA B tensors: loaded mid-kernel (off the hot front)
            uB_sb = cpool.tile([128, HT * 128], bf16, name="uB_sb")
            nc.sync.dma_start(out=uB_sb[:], in_=d_uB[:])
            gB_sb = cpool.tile([128, HT * 128], bf16, name="gB_sb")
            nc.sync.dma_start(out=gB_sb[:], in_=d_gB[:])
            dA_sb = cpool.tile([128, HT * 128], bf16, name="dA_sb")
            nc.sync.dma_start(out=dA_sb[:], in_=d_dA[:])

            # ---- merged loop: full tiles STASH_P..HT-1, each also carrying
            # one stashed tile's deferred LoRA + elementwise ----
            psV = pspool.tile([128, 2 * TC], f32, tag="psV", name="psV")
            pend = []                   # [(tile_idx, ch_pair), ...]
            vstate = {"first": True}

            def flush_pend(final=False):
                for n, (pi, pch) in enumerate(pend):
                    last = final and n == len(pend) - 1
                    nc.tensor.matmul(psV[:],
                                     dA_sb[:, pi * 128:(pi + 1) * 128],
                                     pch[:], start=vstate["first"],
                                     stop=last, skip_group_check=True)
                    vstate["first"] = False
                pend.clear()

            def lora_elemwise(i, ug_src):
                ch_pair = spool.tile([128, 2 * TC], bf16, tag="chp", bufs=4)
                for j in range(2):
                    # psLO_j = [lo_up_j | lo_gate_j], one PSUM bank
                    psLO = pspool.tile([128, 2 * TC], f32, tag="psLO",
                                       bufs=4, name="psLO")
                    nc.tensor.matmul(psLO[:, 0:TC],
                                     uB_sb[:, i * 128:(i + 1) * 128],
                                     Ut[:, j * TC:(j + 1) * TC],
                                     start=True, stop=True)
                    nc.tensor.matmul(psLO[:, TC:2 * TC],
                                     gB_sb[:, i * 128:(i + 1) * 128],
                                     Gt[:, j * TC:(j + 1) * TC],
                                     start=True, stop=True)
                    # [U+lo_u | G+lo_g] in one 512-wide add (DVE reads only
                    # ONE PSUM input per op, so U|G must come from SBUF)
                    tusg = spool.tile([128, 2 * TC], bf16, tag="tusg",
                                      bufs=3)
                    nc.vector.tensor_tensor(tusg[:], ug_src, psLO[:],
                                            AL.add)
                    su = spool.tile([128, TC], bf16, tag="su")
                    nc.scalar.activation(su[:], tusg[:, 0:TC], AF.Silu)
                    hh = spool.tile([128, TC], bf16, tag="hh")
                    nc.vector.tensor_tensor(hh[:], su[:], tusg[:, TC:2 * TC],
                                            AL.mult)
                    nc.vector.tensor_tensor(ch_pair[:, j * TC:(j + 1) * TC],
                                            hh[:],
                                            cb[:, j * TC:(j + 1) * TC],
                                            AL.mult)
                nc.gpsimd.tensor_tensor(mixed[:, i * TC:(i + 1) * TC],
                                        ch_pair[:, 0:TC],
                                        ch_pair[:, TC:2 * TC], AL.add)
                pend.append((i, ch_pair))

            wd_pre = {}

            HW2 = HT * 128 // 2       # 2816 cols per wd half-tile

            def load_wd(di):
                halves = []
                for hf in range(2):
                    t = wpool.tile([128, HW2], bf16, tag="wd", bufs=4)
                    nc.sync.dma_start(
                        out=t[:], in_=d_wd[di][:, hf * HW2:(hf + 1) * HW2])
                    halves.append(t)
                return halves

            for i in range(STASH_P, HT):
                wugh = load_wug(i)
                if i == HT - 1:         # prefetch first down-proj tile
                    wd_pre[0] = load_wd(0)
                psUG = base_chain(i, wugh)
                flush_pend()
                UG_sb = spool.tile([128, 2 * TC], bf16, tag="UG", bufs=3)
                nc.scalar.copy(UG_sb[:], psUG[:])
                lora_elemwise(i, UG_sb[:])
                si = i - STASH_P
                if si < STASH_P:
                    lora_elemwise(
                        si, UGstash[:, si * 2 * TC:(si + 1) * 2 * TC])
            flush_pend(final=True)
            # masked v (one 512-wide mult)
            nc.vector.tensor_tensor(vt[:], psV[:], Mj[:], AL.mult)

            # ---- down GEMM + LoRA-down ----
            dB_sb = cpool.tile([128, D], bf16, name="dB_sb")
            nc.sync.dma_start(out=dB_sb[:], in_=d_dB[:])
            for di in range(DT):
                wd_t = wd_pre.pop(di) if di in wd_pre else load_wd(di)
                psO = pspool.tile([128, TC], f32, tag="psUG", bufs=2,
                                  name="psO")
                for hk in range(HT):
                    w2 = wd_t[hk // 22]
                    nc.tensor.matmul(psO[:],
                                     w2[:, (hk % 22) * 128:(hk % 22 + 1) * 128],
                                     mixed[:, hk * TC:(hk + 1) * TC],
                                     start=(hk == 0), stop=False,
                                     skip_group_check=True)
                nc.tensor.matmul(psO[:], dB_sb[:, di * 128:(di + 1) * 128],
                                 vt[:, 0:TC], start=False, stop=False,
                                 skip_group_check=True)
                nc.tensor.matmul(psO[:], dB_sb[:, di * 128:(di + 1) * 128],
                                 vt[:, TC:2 * TC], start=False, stop=True,
                                 skip_group_check=True)
                o_sb = spool.tile([128, TC], f32, tag="o_sb")
                nc.scalar.copy(o_sb[:], psO[:])
                nc.sync.dma_start(out=d_out[di * 128:(di + 1) * 128, :],
                                  in_=o_sb[:])

    nc.compile()
    return nc


def _prep_shared(inputs):
    """Host-side layout prep of weight tensors (shared across cores)."""
    import ml_dtypes
    bf16 = np.dtype(ml_dtypes.bfloat16)
    f32 = np.float32

    def c(a, dt):
        return np.ascontiguousarray(a.astype(dt, copy=False))

    w_up, w_gate, w_down = inputs["w_up"], inputs["w_gate"], inputs["w_down"]
    wu = (w_up.reshape(HT, 128, KT, 128).transpose(0, 3, 2, 1)
          .reshape(HT, 128, KT * 128))
    wg = (w_gate.reshape(HT, 128, KT, 128).transpose(0, 3, 2, 1)
          .reshape(HT, 128, KT * 128))
    wug = c(np.concatenate([wu, wg], axis=2), bf16)
    wd = c(w_down.reshape(DT, 128, HT, 128).transpose(0, 3, 2, 1)
           .reshape(DT, 128, HT * 128), bf16)

    A_stack = np.concatenate([
        inputs["up_A"].reshape(ER, D),
        inputs["gate_A"].reshape(ER, D)], axis=0)          # [2*ER, D]
    # Ah[p, k*2ER + m] = A_stack[m, k*128+p]
    Ah = c(A_stack.reshape(2 * ER, KT, 128).transpose(2, 1, 0)
           .reshape(128, KT * 2 * ER), bf16)

    # resident B tensors: [er=128 partitions, h cols] row-major
    up_B_all = (inputs["up_B"].transpose(0, 2, 1).reshape(ER, H)
                * ALPHA).astype(f32)
    gate_B_all = (inputs["gate_B"].transpose(0, 2, 1).reshape(ER, H)
                  * ALPHA).astype(f32)
    uB = c(up_B_all, bf16)                                 # [128, H]
    gB = c(gate_B_all, bf16)
    down_A_all = inputs["down_A"].reshape(ER, H).astype(f32)
    # dA[p, i*128+er] = down_A[er, i*128+p]  (lhsT [h-part, er-free])
    dA = c(down_A_all.T.reshape(HT, 128, ER).transpose(1, 0, 2)
           .reshape(128, HT * ER), bf16)
    down_B_all = (inputs["down_B"].transpose(0, 2, 1).reshape(ER, D)
                  * ALPHA).astype(f32)
    dB = c(down_B_all, bf16)

    gate_wT = inputs["gate_w"].T.astype(f32)               # [D, E]
    gw = c(gate_wT.reshape(KT, 128, E).transpose(1, 0, 2)
           .reshape(128, KT * E), f32)
    gwb = c(gw, bf16)

    eid = (8.0 - (np.arange(128) // R)).astype(f32).reshape(128, 1)
    i8m = np.tile((8.0 - np.arange(E)).astype(f32), (128, 1))
    sel2 = np.zeros((2, 256), f32)
    sel2[0, 0:128] = 1.0
    sel2[1, 128:256] = 1.0

    return dict(wug=wug, wd=wd, Ah=Ah, uB=uB, gB=gB, dA=dA, dB=dB,
                gw=gw, gwb=gwb, eid=eid, i8m=i8m, sel2=sel2)


def _in_maps(inputs):
    """Build per-core input maps (shared weights + per-core x slices)."""
    import ml_dtypes
    bf16 = np.dtype(ml_dtypes.bfloat16)
    shared = _prep_shared(inputs)
    x = np.asarray(inputs["x"]).astype(np.float32)
    xt = x.reshape(T, D)

    maps = []
    for cix in range(NCORES):
        b = (cix * TC) // S
        o = (cix * TC) % S                                 # offset in batch
        xb = xt[b * S:(b + 1) * S]                         # [S, D] batch
        rolled = np.concatenate([xb[o:], xb[:o]], axis=0)  # local 256 first
        m = dict(shared)
        # prepacked SBUF layouts: [128, k*W + t] = x.T[k*128+p, t]
        loc = np.ascontiguousarray(
            rolled[0:TC].T.reshape(KT, 128, TC).transpose(1, 0, 2)
            .reshape(128, KT * TC))                        # f32
        m["xL"] = loc
        m["xT"] = loc.astype(bf16)
        m["xR"] = np.ascontiguousarray(
            rolled[TC:].T.astype(bf16).reshape(KT, 128, SR)
            .transpose(1, 0, 2).reshape(128, KT * SR))
        maps.append(m)
    return maps


def kernel(**inputs):
    from concourse.bass_utils import run_bass_kernel_spmd

    inputs = {k: np.asarray(v) for k, v in inputs.items()}
    if "nc" not in _cache:
        _cache["nc"] = _build()
    nc = _cache["nc"]

    in_maps = _in_maps(inputs)
    res = run_bass_kernel_spmd(nc, in_maps, list(range(NCORES)))
    out = np.empty((T, D), np.float32)
    for cix in range(NCORES):
        out[cix * TC:(cix + 1) * TC, :] = res.results[cix]["outT"].T
    return out.reshape(B, S, D)


# revision 17
# speedup vs baseline: 1.3537x; 1.0097x over previous
"""Trainium2 Bass kernel for nn_MistralMoLoraLayer (MoE-routed LoRA FFN).

Strategy: data-parallel over tokens (8 cores x 256 tokens), base FFN weights
replicated in bf16, all-expert LoRA replicated (resident in SBUF, bf16).

No collectives: the per-(batch,slot) softmax over the sequence axis needs
global denominators, so each core redundantly computes the router (logits +
top-2 + exp) for its WHOLE batch (1024 tokens) and sums the denominators
locally (an AllReduce measured ~0.8 ms/call of sync overhead on hw). The
batch tokens are rolled per-core on the host so each core's own 256 tokens
land in positions 0:256 -> the program is SPMD-identical across cores. The
local 256 tokens' logits are computed in f32 (they pick experts + weights);
the other 768 feed only the denominator sum, where bf16 rounding averages
out, so they stream as bf16 to cut front-of-kernel DMA pressure.

Schedule: the first STASH_P h-tiles' base GEMMs run BEFORE the router's
matmuls in PE program order (staging U|G to SBUF) so the PE stays busy
while the router token stream is in flight; their LoRA + elementwise are
deferred and interleaved one-per-tile into the main loop. SBUF-only
multiplies run on the otherwise-idle GpSimd engine to keep DVE under the
PE pace.

Per-core math (all tiles [h/er/d partitions, tokens free]):
  router: logits = xB @ gate_w.T for the 1024 batch tokens; top-2
          (value,index) per token; exp; local denominator sum; weights
          w_j = exp_j / denom[slot j] for the local 256 tokens
  A-proj: UA/GA [E*R=128, t] = stacked up_A/gate_A @ x.T
  slot-mask trick: Ut_j = UA * M_j where M_j[e*R+r, t] = (sel_j(t)==e);
          lo_up_j[h,t] = (stacked up_B) @ Ut_j  == up_B[sel_j(t)] @ u_sel
  h_j = silu(U + lo_up_j) * (G + lo_gate_j); ch_j = c_j * h_j
  mixed = ch_0 + ch_1
  v_j[er,t] = (stacked down_A) @ ch_j  (accumulated over h), masked by M_j
  outT[d,t] = w_down-chain @ mixed + (stacked down_B) @ v_0 + ... @ v_1
"""

import numpy as np

# problem constants (hardcoded; kernel.py must be self-contained)
B, S, D, H, E, R, TOPK = 2, 1024, 2048, 5632, 8, 16, 2
ALPHA = 2.0
T = B * S
NCORES = 8
TC = T // NCORES           # 256 tokens per core
SB = S                     # batch tokens seen by the router (1024)
SR = SB - TC               # non-local batch tokens (768)
KT = D // 128              # 16 k-tiles over D
HT = H // 128              # 44 h-tiles
DT = D // 128              # 16 d-tiles
ER = E * R                 # 128
STASH_P = 16               # h-tiles whose base GEMM runs before the router

SKIP_AR = False            # kept for test.py compat; no collective anymore

_cache = {}


def _build():
    import concourse.bacc as bacc
    import concourse.bass as bass
    import concourse.mybir as mybir
    import concourse.tile as tile
    from concourse.masks import make_identity

    f32 = mybir.dt.float32
    bf16 = mybir.dt.bfloat16
    AL = mybir.AluOpType
    AF = mybir.ActivationFunctionType

    nc = bacc.Bacc("TRN2", target_bir_lowering=False, debug=False,
                   num_devices=NCORES)

    # ---- DRAM I/O ----
    d_xT = nc.dram_tensor("xT", [128, KT * TC], bf16,
                          kind="ExternalInput").ap()
    d_xD = nc.dram_tensor("xD", [128, KT * TC], bf16,
                          kind="ExternalInput").ap()
    d_xR = nc.dram_tensor("xR", [128, KT * SR], bf16,
                          kind="ExternalInput").ap()
    d_gwb = nc.dram_tensor("gwb", [128, KT * E], bf16,
                           kind="ExternalInput").ap()
    d_gwd = nc.dram_tensor("gwd", [128, KT * E], bf16,
                           kind="ExternalInput").ap()
    d_wug = nc.dram_tensor("wug", [HT, 128, 2 * KT * 128], bf16,
                           kind="ExternalInput").ap()
    d_wd = nc.dram_tensor("wd", [DT, 128, HT * 128], bf16,
                          kind="ExternalInput").ap()
    d_A = nc.dram_tensor("Ah", [128, KT * 2 * ER], bf16,
                         kind="ExternalInput").ap()
    d_uB = nc.dram_tensor("uB", [128, HT * 128], bf16,
                          kind="ExternalInput").ap()
    d_gB = nc.dram_tensor("gB", [128, HT * 128], bf16,
                          kind="ExternalInput").ap()
    d_dA = nc.dram_tensor("dA", [128, HT * 128], bf16,
                          kind="ExternalInput").ap()
    d_dB = nc.dram_tensor("dB", [128, D], bf16, kind="ExternalInput").ap()
    d_eid = nc.dram_tensor("eid", [128, 1], f32, kind="ExternalInput").ap()
    d_i8m = nc.dram_tensor("i8m", [128, E], f32, kind="ExternalInput").ap()
    d_sel2 = nc.dram_tensor("sel2", [2, 256], f32, kind="ExternalInput").ap()
    d_out = nc.dram_tensor("outT", [D, TC], f32, kind="ExternalOutput").ap()

    with tile.TileContext(nc) as tc:
        import contextlib
        ctx = contextlib.ExitStack()
        with ctx:
            cpool = ctx.enter_context(tc.tile_pool(name="const", bufs=1))
            wpool = ctx.enter_context(tc.tile_pool(name="wstream", bufs=2))
            spool = ctx.enter_context(tc.tile_pool(name="work", bufs=2))
            pspool = ctx.enter_context(
                tc.tile_pool(name="ps", bufs=1, space="PSUM"))

            # ---- xT first: everything needs it; other consts interleave
            # into pass A so the first base chain starts ASAP ----
            xT_sb = cpool.tile([128, KT * TC], bf16, name="xT_sb")
            XH = KT * TC // 2
            nc.sync.dma_start(out=xT_sb[:, 0:XH], in_=d_xT[:, 0:XH])
            nc.sync.dma_start(out=xT_sb[:, XH:], in_=d_xT[:, XH:])
            gwb_sb = cpool.tile([128, KT * E], bf16, name="gwb_sb")
            gwd_sb = cpool.tile([128, KT * E], bf16, name="gwd_sb")
            eid_sb = cpool.tile([128, 1], f32, name="eid_sb")
            i8m_sb = cpool.tile([128, E], f32, name="i8m_sb")
            sel2_sb = cpool.tile([2, 256], f32, name="sel2_sb")
            A_sb = cpool.tile([128, KT * 2 * ER], bf16, name="A_sb")

            ident = cpool.tile([128, 128], f32, name="ident")
            make_identity(nc, ident)
            ones_col = cpool.tile([128, 1], f32, name="ones_col")
            nc.vector.memset(ones_col, 1.0)

            mixed = cpool.tile([128, HT * TC], bf16, name="mixed")
            ev_rows = cpool.tile([2, TC], f32, name="ev_rows")
            s_rows = cpool.tile([2, TC], f32, name="s_rows")
            crows = cpool.tile([2, TC], f32, name="crows")
            cb = cpool.tile([128, 2 * TC], bf16, name="cb")
            Mj = cpool.tile([128, 2 * TC], bf16, name="Mj")
            UA = cpool.tile([128, TC], bf16, name="UA")
            GA = cpool.tile([128, TC], bf16, name="GA")
            Ut = cpool.tile([128, 2 * TC], bf16, name="Ut")
            Gt = cpool.tile([128, 2 * TC], bf16, name="Gt")
            vt = cpool.tile([128, 2 * TC], bf16, name="vt")
            dacc = cpool.tile([128, 2], f32, name="dacc")
            UGstash = cpool.tile([128, STASH_P * 2 * TC], bf16,
                                 name="UGstash")

            WH = KT * 128             # 2048 cols per wug half (up | gate)

            def load_wug(i):
                t = wpool.tile([128, 2 * KT * 128], bf16, tag="wug", bufs=3)
                nc.sync.dma_start(out=t[:, 0:WH], in_=d_wug[i][:, 0:WH])
                nc.sync.dma_start(out=t[:, WH:], in_=d_wug[i][:, WH:])
                return t

            def base_chain(i, wugh):
                psUG = pspool.tile([128, 2 * TC], f32, tag="psUG", bufs=2,
                                   name="psUG")
                for k in range(KT):
                    nc.tensor.matmul(psUG[:, 0:TC],
                                     wugh[:, k * 128:(k + 1) * 128],
                                     xT_sb[:, k * TC:(k + 1) * TC],
                                     start=(k == 0), stop=(k == KT - 1))
                for k in range(KT):
                    nc.tensor.matmul(psUG[:, TC:2 * TC],
                                     wugh[:, (KT + k) * 128:(KT + k + 1) * 128],
                                     xT_sb[:, k * TC:(k + 1) * TC],
                                     start=(k == 0), stop=(k == KT - 1))
                return psUG

            # ---- pass A: base GEMMs for the first STASH_P tiles, with the
            # router token stream's DMAs interleaved into the queue (one
            # k-slice per pass-A tile, so wu/wg and xL/xR share bandwidth
            # and the first base chain isn't queued behind the full 5.6 MB
            # router stream) ----
            xd_sb = cpool.tile([128, KT * TC], bf16, name="xd_sb")
            xr_sb = cpool.tile([128, KT * SR], bf16, name="xr_sb")
            for i in range(STASH_P):
                wugh = load_wug(i)
                if i == 1:
                    nc.sync.dma_start(out=gwb_sb[:], in_=d_gwb[:])
                    nc.sync.dma_start(out=gwd_sb[:], in_=d_gwd[:])
                elif i == 3:
                    nc.sync.dma_start(out=xd_sb[:], in_=d_xD[:])
                elif i == 5:
                    nc.sync.dma_start(out=xr_sb[:, 0:KT * SR // 2],
                                      in_=d_xR[:, 0:KT * SR // 2])
                elif i == 8:
                    nc.sync.dma_start(out=xr_sb[:, KT * SR // 2:],
                                      in_=d_xR[:, KT * SR // 2:])
                elif i == 11:
                    nc.sync.dma_start(out=A_sb[:], in_=d_A[:])
                elif i == 13:
                    nc.sync.dma_start(out=eid_sb[:], in_=d_eid[:])
                    nc.sync.dma_start(out=i8m_sb[:], in_=d_i8m[:])
                    nc.sync.dma_start(out=sel2_sb[:], in_=d_sel2[:])
                psUG = base_chain(i, wugh)
                nc.scalar.copy(
                    UGstash[:, i * 2 * TC:(i + 1) * 2 * TC], psUG[:])


            # ---- router matmuls (xL/xR fully streamed by now) ----
            # psL pairs [128,512]: two token-tiles' [128,8] logit blocks at
            # cols 0 and 256. ONE start per bank (a second start=True would
            # clear the whole bank's has_written and break the sibling
            # region's accumulation); the h=1 region's first write lands on
            # cleared has_written and correctly overwrites.
            psLp = [pspool.tile([128, 2 * TC], f32, tag="psLO", bufs=4,
                                name=f"psLp{p}") for p in range(4)]
            for k in range(KT):
                for h in range(2):
                    # local logits, f32-exact via bf16 error-splitting:
                    # L = (xb+xd)@(gwb+gwd), dropping the ~2^-18 xd@gwd term
                    for lhs, rhs in ((xT_sb, gwb_sb), (xd_sb, gwb_sb),
                                     (xT_sb, gwd_sb)):
                        nc.tensor.matmul(
                            psLp[0][:, h * TC:h * TC + E],
                            lhs[:, k * TC + h * 128:k * TC + (h + 1) * 128],
                            rhs[:, k * E:(k + 1) * E],
                            start=(k == 0 and h == 0 and lhs is xT_sb
                                   and rhs is gwb_sb),
                            stop=(k == KT - 1 and rhs is gwd_sb),
                            skip_group_check=True)
                for p in range(1, 4):
                    for h in range(2):
                        nc.tensor.matmul(
                            psLp[p][:, h * TC:h * TC + E],
                            xr_sb[:, k * SR + (2 * p + h - 2) * 128:
                                  k * SR + (2 * p + h - 1) * 128],
                            gwb_sb[:, k * E:(k + 1) * E],
                            start=(k == 0 and h == 0), stop=(k == KT - 1),
                            skip_group_check=(h == 1))

            for tt in range(2 * 4):
                psL = psLp[tt // 2][:, (tt % 2) * TC:(tt % 2) * TC + E]
                L = spool.tile([128, E], f32, tag="L", bufs=3)
                nc.vector.tensor_copy(L[:], psL)
                mx1 = spool.tile([128, 1], f32, tag="mx1")
                nc.vector.tensor_reduce(mx1[:], L[:], mybir.AxisListType.X,
                                        AL.max)
                msk = spool.tile([128, E], f32, tag="msk")
                nc.vector.tensor_scalar(msk[:], L[:], mx1[:], None,
                                        AL.is_equal)
                evals = spool.tile([128, 2], f32, tag="evals", bufs=3)
                nc.scalar.activation(evals[:, 0:1], mx1[:], AF.Exp)
                # mask out slot-0 winner, find second max
                big = spool.tile([128, E], f32, tag="big")
                nc.vector.tensor_scalar(big[:], msk[:], 1e30, None, AL.mult)
                L2 = spool.tile([128, E], f32, tag="L2")
                nc.vector.tensor_tensor(L2[:], L[:], big[:], AL.subtract)
                mx2 = spool.tile([128, 1], f32, tag="mx2")
                nc.vector.tensor_reduce(mx2[:], L2[:], mybir.AxisListType.X,
                                        AL.max)
                nc.scalar.activation(evals[:, 1:2], mx2[:], AF.Exp)
                # denominator accumulation (all 8 tiles)
                if tt == 0:
                    nc.vector.tensor_copy(dacc[:], evals[:])
                else:
                    nc.vector.tensor_tensor(dacc[:], dacc[:], evals[:],
                                            AL.add)
                if tt < 2:
                    # local tokens: need expert ids + value rows
                    svals = spool.tile([128, 2], f32, tag="svals")
                    mi = spool.tile([128, E], f32, tag="mi")
                    nc.vector.tensor_tensor(mi[:], msk[:], i8m_sb[:], AL.mult)
                    nc.vector.tensor_reduce(svals[:, 0:1], mi[:],
                                            mybir.AxisListType.X, AL.max)
                    msk2 = spool.tile([128, E], f32, tag="msk2")
                    nc.vector.tensor_scalar(msk2[:], L2[:], mx2[:], None,
                                            AL.is_equal)
                    mi2 = spool.tile([128, E], f32, tag="mi2")
                    nc.vector.tensor_tensor(mi2[:], msk2[:], i8m_sb[:],
                                            AL.mult)
                    nc.vector.tensor_reduce(svals[:, 1:2], mi2[:],
                                            mybir.AxisListType.X, AL.max)
                    # transpose evals/svals -> rows
                    psT = pspool.tile([2, 128], f32, tag="ps_small",
                                      name="psT")
                    nc.tensor.transpose(psT[:], evals[:], ident[:])
                    nc.vector.tensor_copy(
                        ev_rows[:, tt * 128:(tt + 1) * 128], psT[:])
                    psT2 = pspool.tile([2, 128], f32, tag="ps_small",
                                       name="psT2")
                    nc.tensor.transpose(psT2[:], svals[:], ident[:])
                    nc.vector.tensor_copy(
                        s_rows[:, tt * 128:(tt + 1) * 128], psT2[:])

            # denominators: [2,1] = dacc.T @ ones, reciprocal
            psDC = pspool.tile([2, 1], f32, tag="ps_small", name="psDC")
            nc.tensor.matmul(psDC[:], dacc[:], ones_col[:],
                             start=True, stop=True)
            rcp = cpool.tile([2, 1], f32, name="rcp")
            nc.vector.reciprocal(rcp[:], psDC[:])
            # normalized routing weights as rows [2, TC]
            nc.vector.tensor_scalar(crows[:], ev_rows[:], rcp[:], None,
                                    AL.mult)

            # broadcast slot rows along partitions via K=2 matmul with a
            # row-selector constant (sel2[:, j*128:(j+1)*128] has row j = 1)
            for j in range(2):
                psBr = pspool.tile([128, TC], f32, tag="ps_small",
                                   name="psBr")
                nc.tensor.matmul(psBr[:], sel2_sb[:, j * 128:(j + 1) * 128],
                                 crows[:], start=True, stop=True)
                nc.vector.tensor_copy(cb[:, j * TC:(j + 1) * TC], psBr[:])
                psM = pspool.tile([128, TC], f32, tag="ps_small",
                                  name="psM")
                nc.tensor.matmul(psM[:], sel2_sb[:, j * 128:(j + 1) * 128],
                                 s_rows[:], start=True, stop=True)
                nc.vector.tensor_scalar(Mj[:, j * TC:(j + 1) * TC], psM[:],
                                        eid_sb[:], None, AL.is_equal)

            # ---- A-proj (needs only xT+A; results used post-router) ----
            psUA = pspool.tile([128, TC], f32, tag="psUG", bufs=2,
                               name="psUA")
            for k in range(KT):
                nc.tensor.matmul(psUA[:],
                                 A_sb[:, k * 2 * ER: k * 2 * ER + ER],
                                 xT_sb[:, k * TC:(k + 1) * TC],
                                 start=(k == 0), stop=(k == KT - 1))
            nc.vector.tensor_copy(UA[:], psUA[:])
            psGA = pspool.tile([128, TC], f32, tag="psUG", bufs=2,
                               name="psGA")
            for k in range(KT):
                nc.tensor.matmul(psGA[:],
                                 A_sb[:, k * 2 * ER + ER:(k + 1) * 2 * ER],
                                 xT_sb[:, k * TC:(k + 1) * TC],
                                 start=(k == 0), stop=(k == KT - 1))
            nc.vector.tensor_copy(GA[:], psGA[:])

            for j in range(2):
                nc.vector.tensor_tensor(Ut[:, j * TC:(j + 1) * TC], UA[:],
                                        Mj[:, j * TC:(j + 1) * TC], AL.mult)
                nc.vector.tensor_tensor(Gt[:, j * TC:(j + 1) * TC], GA[:],
                                        Mj[:, j * TC:(j + 1) * TC], AL.mult)

            # resident LoRA B tensors: loaded mid-kernel (off the hot front)
            uB_sb = cpool.tile([128, HT * 128], bf16, name="uB_sb")
            nc.sync.dma_start(out=uB_sb[:], in_=d_uB[:])
            gB_sb = cpool.tile([128, HT * 128], bf16, name="gB_sb")
            nc.sync.dma_start(out=gB_sb[:], in_=d_gB[:])
            dA_sb = cpool.tile([128, HT * 128], bf16, name="dA_sb")
            nc.sync.dma_start(out=dA_sb[:], in_=d_dA[:])

            # ---- merged loop: full tiles STASH_P..HT-1, each also carrying
            # one stashed tile's deferred LoRA + elementwise ----
            psV = pspool.tile([128, 2 * TC], f32, tag="psV", name="psV")
            pend = []                   # [(tile_idx, ch_pair), ...]
            vstate = {"first": True}

            def flush_pend(final=False):
                for n, (pi, pch) in enumerate(pend):
                    last = final and n == len(pend) - 1
                    nc.tensor.matmul(psV[:],
                                     dA_sb[:, pi * 128:(pi + 1) * 128],
                                     pch[:], start=vstate["first"],
                                     stop=last, skip_group_check=True)
                    vstate["first"] = False
                pend.clear()

            def lora_elemwise(i, ug_src):
                ch_pair = spool.tile([128, 2 * TC], bf16, tag="chp", bufs=4)
                for j in range(2):
                    # psLO_j = [lo_up_j | lo_gate_j], one PSUM bank
                    psLO = pspool.tile([128, 2 * TC], f32, tag="psLO",
                                       bufs=4, name="psLO")
                    nc.tensor.matmul(psLO[:, 0:TC],
                                     uB_sb[:, i * 128:(i + 1) * 128],
                                     Ut[:, j * TC:(j + 1) * TC],
                                     start=True, stop=True)
                    nc.tensor.matmul(psLO[:, TC:2 * TC],
                                     gB_sb[:, i * 128:(i + 1) * 128],
                                     Gt[:, j * TC:(j + 1) * TC],
                                     start=True, stop=True)
                    # [U+lo_u | G+lo_g] in one 512-wide add (DVE reads only
                    # ONE PSUM input per op, so U|G must come from SBUF)
                    tusg = spool.tile([128, 2 * TC], bf16, tag="tusg",
                                      bufs=3)
                    nc.vector.tensor_tensor(tusg[:], ug_src, psLO[:],
                                            AL.add)
                    su = spool.tile([128, TC], bf16, tag="su")
                    nc.scalar.activation(su[:], tusg[:, 0:TC], AF.Silu)
                    hh = spool.tile([128, TC], bf16, tag="hh")
                    nc.vector.tensor_tensor(hh[:], su[:], tusg[:, TC:2 * TC],
                                            AL.mult)
                    nc.vector.tensor_tensor(ch_pair[:, j * TC:(j + 1) * TC],
                                            hh[:],
                                            cb[:, j * TC:(j + 1) * TC],
                                            AL.mult)
                nc.gpsimd.tensor_tensor(mixed[:, i * TC:(i + 1) * TC],
                                        ch_pair[:, 0:TC],
                                        ch_pair[:, TC:2 * TC], AL.add)
                pend.append((i, ch_pair))

            wd_pre = {}

            HW2 = HT * 128 // 2       # 2816 cols per wd half-tile

            def load_wd(di):
                halves = []
                for hf in range(2):
                    t = wpool.tile([128, HW2], bf16, tag="wd", bufs=4)
                    nc.sync.dma_start(
                        out=t[:], in_=d_wd[di][:, hf * HW2:(hf + 1) * HW2])
                    halves.append(t)
                return halves

            for i in range(STASH_P, HT):
                wugh = load_wug(i)
                if i == HT - 1:         # prefetch first down-proj tile
                    wd_pre[0] = load_wd(0)
                psUG = base_chain(i, wugh)
                flush_pend()
                UG_sb = spool.tile([128, 2 * TC], bf16, tag="UG", bufs=3)
                nc.scalar.copy(UG_sb[:], psUG[:])
                lora_elemwise(i, UG_sb[:])
                si = i - STASH_P
                if si < STASH_P:
                    lora_elemwise(
                        si, UGstash[:, si * 2 * TC:(si + 1) * 2 * TC])
            flush_pend(final=True)
            # masked v (one 512-wide mult)
            nc.vector.tensor_tensor(vt[:], psV[:], Mj[:], AL.mult)

            # ---- down GEMM + LoRA-down ----
            dB_sb = cpool.tile([128, D], bf16, name="dB_sb")
            nc.sync.dma_start(out=dB_sb[:], in_=d_dB[:])
            for di in range(DT):
                wd_t = wd_pre.pop(di) if di in wd_pre else load_wd(di)
                psO = pspool.tile([128, TC], f32, tag="psUG", bufs=2,
                                  name="psO")
                for hk in range(HT):
                    w2 = wd_t[hk // 22]
                    nc.tensor.matmul(psO[:],
                                     w2[:, (hk % 22) * 128:(hk % 22 + 1) * 128],
                                     mixed[:, hk * TC:(hk + 1) * TC],
                                     start=(hk == 0), stop=False,
                                     skip_group_check=True)
                nc.tensor.matmul(psO[:], dB_sb[:, di * 128:(di + 1) * 128],
                                 vt[:, 0:TC], start=False, stop=False,
                                 skip_group_check=True)
                nc.tensor.matmul(psO[:], dB_sb[:, di * 128:(di + 1) * 128],
                                 vt[:, TC:2 * TC], start=False, stop=True,
                                 skip_group_check=True)
                o_sb = spool.tile([128, TC], f32, tag="o_sb")
                nc.scalar.copy(o_sb[:], psO[:])
                nc.sync.dma_start(out=d_out[di * 128:(di + 1) * 128, :],
                                  in_=o_sb[:])

    nc.compile()
    return nc


def _prep_shared(inputs):
    """Host-side layout prep of weight tensors (shared across cores)."""
    import ml_dtypes
    bf16 = np.dtype(ml_dtypes.bfloat16)
    f32 = np.float32

    def c(a, dt):
        return np.ascontiguousarray(a.astype(dt, copy=False))

    w_up, w_gate, w_down = inputs["w_up"], inputs["w_gate"], inputs["w_down"]
    wu = (w_up.reshape(HT, 128, KT, 128).transpose(0, 3, 2, 1)
          .reshape(HT, 128, KT * 128))
    wg = (w_gate.reshape(HT, 128, KT, 128).transpose(0, 3, 2, 1)
          .reshape(HT, 128, KT * 128))
    wug = c(np.concatenate([wu, wg], axis=2), bf16)
    wd = c(w_down.reshape(DT, 128, HT, 128).transpose(0, 3, 2, 1)
           .reshape(DT, 128, HT * 128), bf16)

    A_stack = np.concatenate([
        inputs["up_A"].reshape(ER, D),
        inputs["gate_A"].reshape(ER, D)], axis=0)          # [2*ER, D]
    # Ah[p, k*2ER + m] = A_stack[m, k*128+p]
    Ah = c(A_stack.reshape(2 * ER, KT, 128).transpose(2, 1, 0)
           .reshape(128, KT * 2 * ER), bf16)

    # resident B tensors: [er=128 partitions, h cols] row-major
    up_B_all = (inputs["up_B"].transpose(0, 2, 1).reshape(ER, H)
                * ALPHA).astype(f32)
    gate_B_all = (inputs["gate_B"].transpose(0, 2, 1).reshape(ER, H)
                  * ALPHA).astype(f32)
    uB = c(up_B_all, bf16)                                 # [128, H]
    gB = c(gate_B_all, bf16)
    down_A_all = inputs["down_A"].reshape(ER, H).astype(f32)
    # dA[p, i*128+er] = down_A[er, i*128+p]  (lhsT [h-part, er-free])
    dA = c(down_A_all.T.reshape(HT, 128, ER).transpose(1, 0, 2)
           .reshape(128, HT * ER), bf16)
    down_B_all = (inputs["down_B"].transpose(0, 2, 1).reshape(ER, D)
                  * ALPHA).astype(f32)
    dB = c(down_B_all, bf16)

    gate_wT = inputs["gate_w"].T.astype(f32)               # [D, E]
    gw = c(gate_wT.reshape(KT, 128, E).transpose(1, 0, 2)
           .reshape(128, KT * E), f32)
    gwb = c(gw, bf16)
    gwd = c(gw - gwb.astype(f32), bf16)

    eid = (8.0 - (np.arange(128) // R)).astype(f32).reshape(128, 1)
    i8m = np.tile((8.0 - np.arange(E)).astype(f32), (128, 1))
    sel2 = np.zeros((2, 256), f32)
    sel2[0, 0:128] = 1.0
    sel2[1, 128:256] = 1.0

    return dict(wug=wug, wd=wd, Ah=Ah, uB=uB, gB=gB, dA=dA, dB=dB,
                gwb=gwb, gwd=gwd, eid=eid, i8m=i8m, sel2=sel2)


def _in_maps(inputs):
    """Build per-core input maps (shared weights + per-core x slices)."""
    import ml_dtypes
    bf16 = np.dtype(ml_dtypes.bfloat16)
    shared = _prep_shared(inputs)
    x = np.asarray(inputs["x"]).astype(np.float32)
    xt = x.reshape(T, D)

    maps = []
    for cix in range(NCORES):
        b = (cix * TC) // S
        o = (cix * TC) % S                                 # offset in batch
        xb = xt[b * S:(b + 1) * S]                         # [S, D] batch
        rolled = np.concatenate([xb[o:], xb[:o]], axis=0)  # local 256 first
        m = dict(shared)
        # prepacked SBUF layouts: [128, k*W + t] = x.T[k*128+p, t]
        loc = np.ascontiguousarray(
            rolled[0:TC].T.reshape(KT, 128, TC).transpose(1, 0, 2)
            .reshape(128, KT * TC))                        # f32
        xb = loc.astype(bf16)
        m["xT"] = xb
        m["xD"] = np.ascontiguousarray((loc - xb.astype(np.float32))
                                       .astype(bf16))
        m["xR"] = np.ascontiguousarray(
            rolled[TC:].T.astype(bf16).reshape(KT, 128, SR)
            .transpose(1, 0, 2).reshape(128, KT * SR))
        maps.append(m)
    return maps


def kernel(**inputs):
    from concourse.bass_utils import run_bass_kernel_spmd

    inputs = {k: np.asarray(v) for k, v in inputs.items()}
    if "nc" not in _cache:
        _cache["nc"] = _build()
    nc = _cache["nc"]

    in_maps = _in_maps(inputs)
    res = run_bass_kernel_spmd(nc, in_maps, list(range(NCORES)))
    out = np.empty((T, D), np.float32)
    for cix in range(NCORES):
        out[cix * TC:(cix + 1) * TC, :] = res.results[cix]["outT"].T
    return out.reshape(B, S, D)


# revision 18
# speedup vs baseline: 1.3661x; 1.0091x over previous
"""Trainium2 Bass kernel for nn_MistralMoLoraLayer (MoE-routed LoRA FFN).

Strategy: data-parallel over tokens (8 cores x 256 tokens), base FFN weights
replicated in bf16, all-expert LoRA replicated (resident in SBUF, bf16).

No collectives: the per-(batch,slot) softmax over the sequence axis needs
global denominators, so each core redundantly computes the router (logits +
top-2 + exp) for its WHOLE batch (1024 tokens) and sums the denominators
locally (an AllReduce measured ~0.8 ms/call of sync overhead on hw). The
batch tokens are rolled per-core on the host so each core's own 256 tokens
land in positions 0:256 -> the program is SPMD-identical across cores. The
local 256 tokens' logits are computed in f32 (they pick experts + weights);
the other 768 feed only the denominator sum, where bf16 rounding averages
out, so they stream as bf16 to cut front-of-kernel DMA pressure.

Schedule: the first STASH_P h-tiles' base GEMMs run BEFORE the router's
matmuls in PE program order (staging U|G to SBUF) so the PE stays busy
while the router token stream is in flight; their LoRA + elementwise are
deferred and interleaved one-per-tile into the main loop. SBUF-only
multiplies run on the otherwise-idle GpSimd engine to keep DVE under the
PE pace.

Per-core math (all tiles [h/er/d partitions, tokens free]):
  router: logits = xB @ gate_w.T for the 1024 batch tokens; top-2
          (value,index) per token; exp; local denominator sum; weights
          w_j = exp_j / denom[slot j] for the local 256 tokens
  A-proj: UA/GA [E*R=128, t] = stacked up_A/gate_A @ x.T
  slot-mask trick: Ut_j = UA * M_j where M_j[e*R+r, t] = (sel_j(t)==e);
          lo_up_j[h,t] = (stacked up_B) @ Ut_j  == up_B[sel_j(t)] @ u_sel
  h_j = silu(U + lo_up_j) * (G + lo_gate_j); ch_j = c_j * h_j
  mixed = ch_0 + ch_1
  v_j[er,t] = (stacked down_A) @ ch_j  (accumulated over h), masked by M_j
  outT[d,t] = w_down-chain @ mixed + (stacked down_B) @ v_0 + ... @ v_1
"""

import numpy as np

# problem constants (hardcoded; kernel.py must be self-contained)
B, S, D, H, E, R, TOPK = 2, 1024, 2048, 5632, 8, 16, 2
ALPHA = 2.0
T = B * S
NCORES = 8
TC = T // NCORES           # 256 tokens per core
SB = S                     # batch tokens seen by the router (1024)
SR = SB - TC               # non-local batch tokens (768)
KT = D // 128              # 16 k-tiles over D
HT = H // 128              # 44 h-tiles
DT = D // 128              # 16 d-tiles
ER = E * R                 # 128
STASH_P = 16               # h-tiles whose base GEMM runs before the router

SKIP_AR = False            # kept for test.py compat; no collective anymore

_cache = {}


def _build():
    import concourse.bacc as bacc
    import concourse.bass as bass
    import concourse.mybir as mybir
    import concourse.tile as tile
    from concourse.masks import make_identity

    f32 = mybir.dt.float32
    bf16 = mybir.dt.bfloat16
    AL = mybir.AluOpType
    AF = mybir.ActivationFunctionType

    nc = bacc.Bacc("TRN2", target_bir_lowering=False, debug=False,
                   num_devices=NCORES)

    # ---- DRAM I/O ----
    d_xT = nc.dram_tensor("xT", [128, KT * TC], bf16,
                          kind="ExternalInput").ap()
    d_xD = nc.dram_tensor("xD", [128, KT * TC], bf16,
                          kind="ExternalInput").ap()
    d_xR = nc.dram_tensor("xR", [128, KT * SR], bf16,
                          kind="ExternalInput").ap()
    d_gwb = nc.dram_tensor("gwb", [128, KT * E], bf16,
                           kind="ExternalInput").ap()
    d_gwd = nc.dram_tensor("gwd", [128, KT * E], bf16,
                           kind="ExternalInput").ap()
    d_wug = nc.dram_tensor("wug", [HT, 128, 2 * KT * 128], bf16,
                           kind="ExternalInput").ap()
    d_wd = nc.dram_tensor("wd", [DT, 128, HT * 128], bf16,
                          kind="ExternalInput").ap()
    d_A = nc.dram_tensor("Ah", [128, KT * 2 * ER], bf16,
                         kind="ExternalInput").ap()
    d_uB = nc.dram_tensor("uB", [128, HT * 128], bf16,
                          kind="ExternalInput").ap()
    d_gB = nc.dram_tensor("gB", [128, HT * 128], bf16,
                          kind="ExternalInput").ap()
    d_dA = nc.dram_tensor("dA", [128, HT * 128], bf16,
                          kind="ExternalInput").ap()
    d_dB = nc.dram_tensor("dB", [128, D], bf16, kind="ExternalInput").ap()
    d_eid = nc.dram_tensor("eid", [128, 1], f32, kind="ExternalInput").ap()
    d_i8m = nc.dram_tensor("i8m", [128, E], f32, kind="ExternalInput").ap()
    d_sel2 = nc.dram_tensor("sel2", [2, 256], f32, kind="ExternalInput").ap()
    d_out = nc.dram_tensor("outT", [D, TC], f32, kind="ExternalOutput").ap()

    with tile.TileContext(nc) as tc:
        import contextlib
        ctx = contextlib.ExitStack()
        with ctx:
            cpool = ctx.enter_context(tc.tile_pool(name="const", bufs=1))
            wpool = ctx.enter_context(tc.tile_pool(name="wstream", bufs=2))
            spool = ctx.enter_context(tc.tile_pool(name="work", bufs=2))
            pspool = ctx.enter_context(
                tc.tile_pool(name="ps", bufs=1, space="PSUM"))

            # ---- xT first: everything needs it; other consts interleave
            # into pass A so the first base chain starts ASAP ----
            xT_sb = cpool.tile([128, KT * TC], bf16, name="xT_sb")
            XH = KT * TC // 2
            XQ = KT * TC // 8
            nc.sync.dma_start(out=xT_sb[:, 0:XQ], in_=d_xT[:, 0:XQ])
            nc.sync.dma_start(out=xT_sb[:, XQ:XH], in_=d_xT[:, XQ:XH])
            nc.sync.dma_start(out=xT_sb[:, XH:], in_=d_xT[:, XH:])
            gwb_sb = cpool.tile([128, KT * E], bf16, name="gwb_sb")
            gwd_sb = cpool.tile([128, KT * E], bf16, name="gwd_sb")
            eid_sb = cpool.tile([128, 1], f32, name="eid_sb")
            i8m_sb = cpool.tile([128, E], f32, name="i8m_sb")
            sel2_sb = cpool.tile([2, 256], f32, name="sel2_sb")
            A_sb = cpool.tile([128, KT * 2 * ER], bf16, name="A_sb")

            ident = cpool.tile([128, 128], f32, name="ident")
            make_identity(nc, ident)
            ones_col = cpool.tile([128, 1], f32, name="ones_col")
            nc.vector.memset(ones_col, 1.0)

            mixed = cpool.tile([128, HT * TC], bf16, name="mixed")
            ev_rows = cpool.tile([2, TC], f32, name="ev_rows")
            s_rows = cpool.tile([2, TC], f32, name="s_rows")
            crows = cpool.tile([2, TC], f32, name="crows")
            cb = cpool.tile([128, 2 * TC], bf16, name="cb")
            Mj = cpool.tile([128, 2 * TC], bf16, name="Mj")
            UA = cpool.tile([128, TC], bf16, name="UA")
            GA = cpool.tile([128, TC], bf16, name="GA")
            Ut = cpool.tile([128, 2 * TC], bf16, name="Ut")
            Gt = cpool.tile([128, 2 * TC], bf16, name="Gt")
            vt = cpool.tile([128, 2 * TC], bf16, name="vt")
            dacc = cpool.tile([128, 2], f32, name="dacc")
            UGstash = cpool.tile([128, STASH_P * 2 * TC], bf16,
                                 name="UGstash")

            WH = KT * 128             # 2048 cols per wug half (up | gate)

            def load_wug(i, first=False):
                t = wpool.tile([128, 2 * KT * 128], bf16, tag="wug", bufs=3)
                if first:
                    # smaller leading slice so the very first base matmul
                    # starts after ~0.25 MB instead of 0.5 MB
                    WQ = WH // 4
                    nc.sync.dma_start(out=t[:, 0:WQ], in_=d_wug[i][:, 0:WQ])
                    nc.sync.dma_start(out=t[:, WQ:WH], in_=d_wug[i][:, WQ:WH])
                else:
                    nc.sync.dma_start(out=t[:, 0:WH], in_=d_wug[i][:, 0:WH])
                nc.sync.dma_start(out=t[:, WH:], in_=d_wug[i][:, WH:])
                return t

            def base_chain(i, wugh):
                psUG = pspool.tile([128, 2 * TC], f32, tag="psUG", bufs=2,
                                   name="psUG")
                for k in range(KT):
                    nc.tensor.matmul(psUG[:, 0:TC],
                                     wugh[:, k * 128:(k + 1) * 128],
                                     xT_sb[:, k * TC:(k + 1) * TC],
                                     start=(k == 0), stop=(k == KT - 1))
                for k in range(KT):
                    nc.tensor.matmul(psUG[:, TC:2 * TC],
                                     wugh[:, (KT + k) * 128:(KT + k + 1) * 128],
                                     xT_sb[:, k * TC:(k + 1) * TC],
                                     start=(k == 0), stop=(k == KT - 1))
                return psUG

            # ---- pass A: base GEMMs for the first STASH_P tiles, with the
            # router token stream's DMAs interleaved into the queue (one
            # k-slice per pass-A tile, so wu/wg and xL/xR share bandwidth
            # and the first base chain isn't queued behind the full 5.6 MB
            # router stream) ----
            xd_sb = cpool.tile([128, KT * TC], bf16, name="xd_sb")
            xr_sb = cpool.tile([128, KT * SR], bf16, name="xr_sb")
            for i in range(STASH_P):
                wugh = load_wug(i, first=(i == 0))
                if i == 1:
                    nc.sync.dma_start(out=gwb_sb[:], in_=d_gwb[:])
                    nc.sync.dma_start(out=gwd_sb[:], in_=d_gwd[:])
                elif i == 3:
                    nc.sync.dma_start(out=xd_sb[:], in_=d_xD[:])
                elif i == 5:
                    nc.sync.dma_start(out=xr_sb[:, 0:KT * SR // 2],
                                      in_=d_xR[:, 0:KT * SR // 2])
                elif i == 8:
                    nc.sync.dma_start(out=xr_sb[:, KT * SR // 2:],
                                      in_=d_xR[:, KT * SR // 2:])
                elif i == 11:
                    nc.sync.dma_start(out=A_sb[:], in_=d_A[:])
                elif i == 13:
                    nc.sync.dma_start(out=eid_sb[:], in_=d_eid[:])
                    nc.sync.dma_start(out=i8m_sb[:], in_=d_i8m[:])
                    nc.sync.dma_start(out=sel2_sb[:], in_=d_sel2[:])
                psUG = base_chain(i, wugh)
                nc.scalar.copy(
                    UGstash[:, i * 2 * TC:(i + 1) * 2 * TC], psUG[:])


            # ---- router matmuls (xL/xR fully streamed by now) ----
            # psL pairs [128,512]: two token-tiles' [128,8] logit blocks at
            # cols 0 and 256. ONE start per bank (a second start=True would
            # clear the whole bank's has_written and break the sibling
            # region's accumulation); the h=1 region's first write lands on
            # cleared has_written and correctly overwrites.
            psLp = [pspool.tile([128, 2 * TC], f32, tag="psLO", bufs=4,
                                name=f"psLp{p}") for p in range(4)]
            for k in range(KT):
                for h in range(2):
                    # local logits, f32-exact via bf16 error-splitting:
                    # L = (xb+xd)@(gwb+gwd), dropping the ~2^-18 xd@gwd term
                    for lhs, rhs in ((xT_sb, gwb_sb), (xd_sb, gwb_sb),
                                     (xT_sb, gwd_sb)):
                        nc.tensor.matmul(
                            psLp[0][:, h * TC:h * TC + E],
                            lhs[:, k * TC + h * 128:k * TC + (h + 1) * 128],
                            rhs[:, k * E:(k + 1) * E],
                            start=(k == 0 and h == 0 and lhs is xT_sb
                                   and rhs is gwb_sb),
                            stop=(k == KT - 1 and rhs is gwd_sb),
                            skip_group_check=True)
                for p in range(1, 4):
                    for h in range(2):
                        nc.tensor.matmul(
                            psLp[p][:, h * TC:h * TC + E],
                            xr_sb[:, k * SR + (2 * p + h - 2) * 128:
                                  k * SR + (2 * p + h - 1) * 128],
                            gwb_sb[:, k * E:(k + 1) * E],
                            start=(k == 0 and h == 0), stop=(k == KT - 1),
                            skip_group_check=(h == 1))

            for tt in range(2 * 4):
                psL = psLp[tt // 2][:, (tt % 2) * TC:(tt % 2) * TC + E]
                L = spool.tile([128, E], f32, tag="L", bufs=3)
                nc.vector.tensor_copy(L[:], psL)
                mx1 = spool.tile([128, 1], f32, tag="mx1")
                nc.vector.tensor_reduce(mx1[:], L[:], mybir.AxisListType.X,
                                        AL.max)
                msk = spool.tile([128, E], f32, tag="msk")
                nc.vector.tensor_scalar(msk[:], L[:], mx1[:], None,
                                        AL.is_equal)
                evals = spool.tile([128, 2], f32, tag="evals", bufs=3)
                nc.scalar.activation(evals[:, 0:1], mx1[:], AF.Exp)
                # mask out slot-0 winner, find second max
                big = spool.tile([128, E], f32, tag="big")
                nc.vector.tensor_scalar(big[:], msk[:], 1e30, None, AL.mult)
                L2 = spool.tile([128, E], f32, tag="L2")
                nc.vector.tensor_tensor(L2[:], L[:], big[:], AL.subtract)
                mx2 = spool.tile([128, 1], f32, tag="mx2")
                nc.vector.tensor_reduce(mx2[:], L2[:], mybir.AxisListType.X,
                                        AL.max)
                nc.scalar.activation(evals[:, 1:2], mx2[:], AF.Exp)
                # denominator accumulation (all 8 tiles)
                if tt == 0:
                    nc.vector.tensor_copy(dacc[:], evals[:])
                else:
                    nc.vector.tensor_tensor(dacc[:], dacc[:], evals[:],
                                            AL.add)
                if tt < 2:
                    # local tokens: need expert ids + value rows
                    svals = spool.tile([128, 2], f32, tag="svals")
                    mi = spool.tile([128, E], f32, tag="mi")
                    nc.vector.tensor_tensor(mi[:], msk[:], i8m_sb[:], AL.mult)
                    nc.vector.tensor_reduce(svals[:, 0:1], mi[:],
                                            mybir.AxisListType.X, AL.max)
                    msk2 = spool.tile([128, E], f32, tag="msk2")
                    nc.vector.tensor_scalar(msk2[:], L2[:], mx2[:], None,
                                            AL.is_equal)
                    mi2 = spool.tile([128, E], f32, tag="mi2")
                    nc.vector.tensor_tensor(mi2[:], msk2[:], i8m_sb[:],
                                            AL.mult)
                    nc.vector.tensor_reduce(svals[:, 1:2], mi2[:],
                                            mybir.AxisListType.X, AL.max)
                    # transpose evals/svals -> rows
                    psT = pspool.tile([2, 128], f32, tag="ps_small",
                                      name="psT")
                    nc.tensor.transpose(psT[:], evals[:], ident[:])
                    nc.vector.tensor_copy(
                        ev_rows[:, tt * 128:(tt + 1) * 128], psT[:])
                    psT2 = pspool.tile([2, 128], f32, tag="ps_small",
                                       name="psT2")
                    nc.tensor.transpose(psT2[:], svals[:], ident[:])
                    nc.vector.tensor_copy(
                        s_rows[:, tt * 128:(tt + 1) * 128], psT2[:])

            # denominators: [2,1] = dacc.T @ ones, reciprocal
            psDC = pspool.tile([2, 1], f32, tag="ps_small", name="psDC")
            nc.tensor.matmul(psDC[:], dacc[:], ones_col[:],
                             start=True, stop=True)
            rcp = cpool.tile([2, 1], f32, name="rcp")
            nc.vector.reciprocal(rcp[:], psDC[:])
            # normalized routing weights as rows [2, TC]
            nc.vector.tensor_scalar(crows[:], ev_rows[:], rcp[:], None,
                                    AL.mult)

            # broadcast slot rows along partitions via K=2 matmul with a
            # row-selector constant (sel2[:, j*128:(j+1)*128] has row j = 1)
            for j in range(2):
                psBr = pspool.tile([128, TC], f32, tag="ps_small",
                                   name="psBr")
                nc.tensor.matmul(psBr[:], sel2_sb[:, j * 128:(j + 1) * 128],
                                 crows[:], start=True, stop=True)
                nc.vector.tensor_copy(cb[:, j * TC:(j + 1) * TC], psBr[:])
                psM = pspool.tile([128, TC], f32, tag="ps_small",
                                  name="psM")
                nc.tensor.matmul(psM[:], sel2_sb[:, j * 128:(j + 1) * 128],
                                 s_rows[:], start=True, stop=True)
                nc.vector.tensor_scalar(Mj[:, j * TC:(j + 1) * TC], psM[:],
                                        eid_sb[:], None, AL.is_equal)

            # ---- A-proj (needs only xT+A; results used post-router) ----
            psUA = pspool.tile([128, TC], f32, tag="psUG", bufs=2,
                               name="psUA")
            for k in range(KT):
                nc.tensor.matmul(psUA[:],
                                 A_sb[:, k * 2 * ER: k * 2 * ER + ER],
                                 xT_sb[:, k * TC:(k + 1) * TC],
                                 start=(k == 0), stop=(k == KT - 1))
            nc.vector.tensor_copy(UA[:], psUA[:])
            psGA = pspool.tile([128, TC], f32, tag="psUG", bufs=2,
                               name="psGA")
            for k in range(KT):
                nc.tensor.matmul(psGA[:],
                                 A_sb[:, k * 2 * ER + ER:(k + 1) * 2 * ER],
                                 xT_sb[:, k * TC:(k + 1) * TC],
                                 start=(k == 0), stop=(k == KT - 1))
            nc.vector.tensor_copy(GA[:], psGA[:])

            for j in range(2):
                nc.vector.tensor_tensor(Ut[:, j * TC:(j + 1) * TC], UA[:],
                                        Mj[:, j * TC:(j + 1) * TC], AL.mult)
                nc.vector.tensor_tensor(Gt[:, j * TC:(j + 1) * TC], GA[:],
                                        Mj[:, j * TC:(j + 1) * TC], AL.mult)

            # resident LoRA B tensors: loaded mid-kernel (off the hot front)
            uB_sb = cpool.tile([128, HT * 128], bf16, name="uB_sb")
            nc.sync.dma_start(out=uB_sb[:], in_=d_uB[:])
            gB_sb = cpool.tile([128, HT * 128], bf16, name="gB_sb")
            nc.sync.dma_start(out=gB_sb[:], in_=d_gB[:])
            dA_sb = cpool.tile([128, HT * 128], bf16, name="dA_sb")
            nc.sync.dma_start(out=dA_sb[:], in_=d_dA[:])

            # ---- merged loop: full tiles STASH_P..HT-1, each also carrying
            # one stashed tile's deferred LoRA + elementwise ----
            psV = pspool.tile([128, 2 * TC], f32, tag="psV", name="psV")
            pend = []                   # [(tile_idx, ch_pair), ...]
            vstate = {"first": True}

            def flush_pend(final=False):
                for n, (pi, pch) in enumerate(pend):
                    last = final and n == len(pend) - 1
                    nc.tensor.matmul(psV[:],
                                     dA_sb[:, pi * 128:(pi + 1) * 128],
                                     pch[:], start=vstate["first"],
                                     stop=last, skip_group_check=True)
                    vstate["first"] = False
                pend.clear()

            def lora_elemwise(i, ug_src):
                ch_pair = spool.tile([128, 2 * TC], bf16, tag="chp", bufs=4)
                for j in range(2):
                    # psLO_j = [lo_up_j | lo_gate_j], one PSUM bank
                    psLO = pspool.tile([128, 2 * TC], f32, tag="psLO",
                                       bufs=4, name="psLO")
                    nc.tensor.matmul(psLO[:, 0:TC],
                                     uB_sb[:, i * 128:(i + 1) * 128],
                                     Ut[:, j * TC:(j + 1) * TC],
                                     start=True, stop=True)
                    nc.tensor.matmul(psLO[:, TC:2 * TC],
                                     gB_sb[:, i * 128:(i + 1) * 128],
                                     Gt[:, j * TC:(j + 1) * TC],
                                     start=True, stop=True)
                    # [U+lo_u | G+lo_g] in one 512-wide add (DVE reads only
                    # ONE PSUM input per op, so U|G must come from SBUF)
                    tusg = spool.tile([128, 2 * TC], bf16, tag="tusg",
                                      bufs=3)
                    nc.vector.tensor_tensor(tusg[:], ug_src, psLO[:],
                                            AL.add)
                    su = spool.tile([128, TC], bf16, tag="su")
                    nc.scalar.activation(su[:], tusg[:, 0:TC], AF.Silu)
                    hh = spool.tile([128, TC], bf16, tag="hh")
                    nc.vector.tensor_tensor(hh[:], su[:], tusg[:, TC:2 * TC],
                                            AL.mult)
                    nc.vector.tensor_tensor(ch_pair[:, j * TC:(j + 1) * TC],
                                            hh[:],
                                            cb[:, j * TC:(j + 1) * TC],
                                            AL.mult)
                nc.gpsimd.tensor_tensor(mixed[:, i * TC:(i + 1) * TC],
                                        ch_pair[:, 0:TC],
                                        ch_pair[:, TC:2 * TC], AL.add)
                pend.append((i, ch_pair))

            wd_pre = {}

            HW2 = HT * 128 // 2       # 2816 cols per wd half-tile

            def load_wd(di):
                halves = []
                for hf in range(2):
                    t = wpool.tile([128, HW2], bf16, tag="wd", bufs=4)
                    nc.sync.dma_start(
                        out=t[:], in_=d_wd[di][:, hf * HW2:(hf + 1) * HW2])
                    halves.append(t)
                return halves

            for i in range(STASH_P, HT):
                wugh = load_wug(i)
                if i == HT - 1:         # prefetch first down-proj tile
                    wd_pre[0] = load_wd(0)
                psUG = base_chain(i, wugh)
                flush_pend()
                UG_sb = spool.tile([128, 2 * TC], bf16, tag="UG", bufs=3)
                nc.scalar.copy(UG_sb[:], psUG[:])
                lora_elemwise(i, UG_sb[:])
                si = i - STASH_P
                if si < STASH_P:
                    lora_elemwise(
                        si, UGstash[:, si * 2 * TC:(si + 1) * 2 * TC])
            flush_pend(final=True)
            # masked v (one 512-wide mult)
            nc.vector.tensor_tensor(vt[:], psV[:], Mj[:], AL.mult)

            # ---- down GEMM + LoRA-down ----
            dB_sb = cpool.tile([128, D], bf16, name="dB_sb")
            nc.sync.dma_start(out=dB_sb[:], in_=d_dB[:])
            for di in range(DT):
                wd_t = wd_pre.pop(di) if di in wd_pre else load_wd(di)
                psO = pspool.tile([128, TC], f32, tag="psUG", bufs=2,
                                  name="psO")
                for hk in range(HT):
                    w2 = wd_t[hk // 22]
                    nc.tensor.matmul(psO[:],
                                     w2[:, (hk % 22) * 128:(hk % 22 + 1) * 128],
                                     mixed[:, hk * TC:(hk + 1) * TC],
                                     start=(hk == 0), stop=False,
                                     skip_group_check=True)
                nc.tensor.matmul(psO[:], dB_sb[:, di * 128:(di + 1) * 128],
                                 vt[:, 0:TC], start=False, stop=False,
                                 skip_group_check=True)
                nc.tensor.matmul(psO[:], dB_sb[:, di * 128:(di + 1) * 128],
                                 vt[:, TC:2 * TC], start=False, stop=True,
                                 skip_group_check=True)
                o_sb = spool.tile([128, TC], f32, tag="o_sb")
                nc.scalar.copy(o_sb[:], psO[:])
                nc.sync.dma_start(out=d_out[di * 128:(di + 1) * 128, :],
                                  in_=o_sb[:])

    nc.compile()
    return nc


def _prep_shared(inputs):
    """Host-side layout prep of weight tensors (shared across cores)."""
    import ml_dtypes
    bf16 = np.dtype(ml_dtypes.bfloat16)
    f32 = np.float32

    def c(a, dt):
        return np.ascontiguousarray(a.astype(dt, copy=False))

    w_up, w_gate, w_down = inputs["w_up"], inputs["w_gate"], inputs["w_down"]
    wu = (w_up.reshape(HT, 128, KT, 128).transpose(0, 3, 2, 1)
          .reshape(HT, 128, KT * 128))
    wg = (w_gate.reshape(HT, 128, KT, 128).transpose(0, 3, 2, 1)
          .reshape(HT, 128, KT * 128))
    wug = c(np.concatenate([wu, wg], axis=2), bf16)
    wd = c(w_down.reshape(DT, 128, HT, 128).transpose(0, 3, 2, 1)
           .reshape(DT, 128, HT * 128), bf16)

    A_stack = np.concatenate([
        inputs["up_A"].reshape(ER, D),
        inputs["gate_A"].reshape(ER, D)], axis=0)          # [2*ER, D]
    # Ah[p, k*2ER + m] = A_stack[m, k*128+p]
    Ah = c(A_stack.reshape(2 * ER, KT, 128).transpose(2, 1, 0)
           .reshape(128, KT * 2 * ER), bf16)

    # resident B tensors: [er=128 partitions, h cols] row-major
    up_B_all = (inputs["up_B"].transpose(0, 2, 1).reshape(ER, H)
                * ALPHA).astype(f32)
    gate_B_all = (inputs["gate_B"].transpose(0, 2, 1).reshape(ER, H)
                  * ALPHA).astype(f32)
    uB = c(up_B_all, bf16)                                 # [128, H]
    gB = c(gate_B_all, bf16)
    down_A_all = inputs["down_A"].reshape(ER, H).astype(f32)
    # dA[p, i*128+er] = down_A[er, i*128+p]  (lhsT [h-part, er-free])
    dA = c(down_A_all.T.reshape(HT, 128, ER).transpose(1, 0, 2)
           .reshape(128, HT * ER), bf16)
    down_B_all = (inputs["down_B"].transpose(0, 2, 1).reshape(ER, D)
                  * ALPHA).astype(f32)
    dB = c(down_B_all, bf16)

    gate_wT = inputs["gate_w"].T.astype(f32)               # [D, E]
    gw = c(gate_wT.reshape(KT, 128, E).transpose(1, 0, 2)
           .reshape(128, KT * E), f32)
    gwb = c(gw, bf16)
    gwd = c(gw - gwb.astype(f32), bf16)

    eid = (8.0 - (np.arange(128) // R)).astype(f32).reshape(128, 1)
    i8m = np.tile((8.0 - np.arange(E)).astype(f32), (128, 1))
    sel2 = np.zeros((2, 256), f32)
    sel2[0, 0:128] = 1.0
    sel2[1, 128:256] = 1.0

    return dict(wug=wug, wd=wd, Ah=Ah, uB=uB, gB=gB, dA=dA, dB=dB,
                gwb=gwb, gwd=gwd, eid=eid, i8m=i8m, sel2=sel2)


def _in_maps(inputs):
    """Build per-core input maps (shared weights + per-core x slices)."""
    import ml_dtypes
    bf16 = np.dtype(ml_dtypes.bfloat16)
    shared = _prep_shared(inputs)
    x = np.asarray(inputs["x"]).astype(np.float32)
    xt = x.reshape(T, D)

    maps = []
    for cix in range(NCORES):
        b = (cix * TC) // S
        o = (cix * TC) % S                                 # offset in batch
        xb = xt[b * S:(b + 1) * S]                         # [S, D] batch
        rolled = np.concatenate([xb[o:], xb[:o]], axis=0)  # local 256 first
        m = dict(shared)
        # prepacked SBUF layouts: [128, k*W + t] = x.T[k*128+p, t]
        loc = np.ascontiguousarray(
            rolled[0:TC].T.reshape(KT, 128, TC).transpose(1, 0, 2)
            .reshape(128, KT * TC))                        # f32
        xb = loc.astype(bf16)
        m["xT"] = xb
        m["xD"] = np.ascontiguousarray((loc - xb.astype(np.float32))
                                       .astype(bf16))
        m["xR"] = np.ascontiguousarray(
            rolled[TC:].T.astype(bf16).reshape(KT, 128, SR)
            .transpose(1, 0, 2).reshape(128, KT * SR))
        maps.append(m)
    return maps


def kernel(**inputs):
    from concourse.bass_utils import run_bass_kernel_spmd

    inputs = {k: np.asarray(v) for k, v in inputs.items()}
    if "nc" not in _cache:
        _cache["nc"] = _build()
    nc = _cache["nc"]

    in_maps = _in_maps(inputs)
    res = run_bass_kernel_spmd(nc, in_maps, list(range(NCORES)))
    out = np.empty((T, D), np.float32)
    for cix in range(NCORES):
        out[cix * TC:(cix + 1) * TC, :] = res.results[cix]["outT"].T
    return out.reshape(B, S, D)


# revision 19
# speedup vs baseline: 1.3766x; 1.0076x over previous
"""Trainium2 Bass kernel for nn_MistralMoLoraLayer (MoE-routed LoRA FFN).

Strategy: data-parallel over tokens (8 cores x 256 tokens), base FFN weights
replicated in bf16, all-expert LoRA replicated (resident in SBUF, bf16).

No collectives: the per-(batch,slot) softmax over the sequence axis needs
global denominators, so each core redundantly computes the router (logits +
top-2 + exp) for its WHOLE batch (1024 tokens) and sums the denominators
locally (an AllReduce measured ~0.8 ms/call of sync overhead on hw). The
batch tokens are rolled per-core on the host so each core's own 256 tokens
land in positions 0:256 -> the program is SPMD-identical across cores. The
local 256 tokens' logits are computed in f32 (they pick experts + weights);
the other 768 feed only the denominator sum, where bf16 rounding averages
out, so they stream as bf16 to cut front-of-kernel DMA pressure.

Schedule: the first STASH_P h-tiles' base GEMMs run BEFORE the router's
matmuls in PE program order (staging U|G to SBUF) so the PE stays busy
while the router token stream is in flight; their LoRA + elementwise are
deferred and interleaved one-per-tile into the main loop. SBUF-only
multiplies run on the otherwise-idle GpSimd engine to keep DVE under the
PE pace.

Per-core math (all tiles [h/er/d partitions, tokens free]):
  router: logits = xB @ gate_w.T for the 1024 batch tokens; top-2
          (value,index) per token; exp; local denominator sum; weights
          w_j = exp_j / denom[slot j] for the local 256 tokens
  A-proj: UA/GA [E*R=128, t] = stacked up_A/gate_A @ x.T
  slot-mask trick: Ut_j = UA * M_j where M_j[e*R+r, t] = (sel_j(t)==e);
          lo_up_j[h,t] = (stacked up_B) @ Ut_j  == up_B[sel_j(t)] @ u_sel
  h_j = silu(U + lo_up_j) * (G + lo_gate_j); ch_j = c_j * h_j
  mixed = ch_0 + ch_1
  v_j[er,t] = (stacked down_A) @ ch_j  (accumulated over h), masked by M_j
  outT[d,t] = w_down-chain @ mixed + (stacked down_B) @ v_0 + ... @ v_1
"""

import numpy as np

# problem constants (hardcoded; kernel.py must be self-contained)
B, S, D, H, E, R, TOPK = 2, 1024, 2048, 5632, 8, 16, 2
ALPHA = 2.0
T = B * S
NCORES = 8
TC = T // NCORES           # 256 tokens per core
SB = S                     # batch tokens seen by the router (1024)
SR = SB - TC               # non-local batch tokens (768)
KT = D // 128              # 16 k-tiles over D
HT = H // 128              # 44 h-tiles
DT = D // 128              # 16 d-tiles
ER = E * R                 # 128
STASH_P = 16               # h-tiles whose base GEMM runs before the router

SKIP_AR = False            # kept for test.py compat; no collective anymore

_cache = {}


def _build():
    import concourse.bacc as bacc
    import concourse.bass as bass
    import concourse.mybir as mybir
    import concourse.tile as tile
    from concourse.masks import make_identity

    f32 = mybir.dt.float32
    bf16 = mybir.dt.bfloat16
    AL = mybir.AluOpType
    AF = mybir.ActivationFunctionType

    nc = bacc.Bacc("TRN2", target_bir_lowering=False, debug=False,
                   num_devices=NCORES)

    # ---- DRAM I/O ----
    d_xT = nc.dram_tensor("xT", [128, KT * TC], bf16,
                          kind="ExternalInput").ap()
    d_xD = nc.dram_tensor("xD", [128, KT * TC], bf16,
                          kind="ExternalInput").ap()
    d_xR = nc.dram_tensor("xR", [128, KT * SR], bf16,
                          kind="ExternalInput").ap()
    d_gwb = nc.dram_tensor("gwb", [128, KT * E], bf16,
                           kind="ExternalInput").ap()
    d_gwd = nc.dram_tensor("gwd", [128, KT * E], bf16,
                           kind="ExternalInput").ap()
    d_wug = nc.dram_tensor("wug", [HT, 128, 2 * KT * 128], bf16,
                           kind="ExternalInput").ap()
    d_wd = nc.dram_tensor("wd", [DT, 128, HT * 128], bf16,
                          kind="ExternalInput").ap()
    d_A = nc.dram_tensor("Ah", [128, KT * 2 * ER], bf16,
                         kind="ExternalInput").ap()
    d_uB = nc.dram_tensor("uB", [128, HT * 128], bf16,
                          kind="ExternalInput").ap()
    d_gB = nc.dram_tensor("gB", [128, HT * 128], bf16,
                          kind="ExternalInput").ap()
    d_dA = nc.dram_tensor("dA", [128, HT * 128], bf16,
                          kind="ExternalInput").ap()
    d_dB = nc.dram_tensor("dB", [128, D], bf16, kind="ExternalInput").ap()
    d_eid = nc.dram_tensor("eid", [128, 1], f32, kind="ExternalInput").ap()
    d_i8m = nc.dram_tensor("i8m", [128, E], f32, kind="ExternalInput").ap()
    d_sel2 = nc.dram_tensor("sel2", [2, 256], f32, kind="ExternalInput").ap()
    d_out = nc.dram_tensor("outT", [D, TC], f32, kind="ExternalOutput").ap()

    with tile.TileContext(nc) as tc:
        import contextlib
        ctx = contextlib.ExitStack()
        with ctx:
            cpool = ctx.enter_context(tc.tile_pool(name="const", bufs=1))
            wpool = ctx.enter_context(tc.tile_pool(name="wstream", bufs=2))
            spool = ctx.enter_context(tc.tile_pool(name="work", bufs=2))
            pspool = ctx.enter_context(
                tc.tile_pool(name="ps", bufs=1, space="PSUM"))

            # ---- first base matmul needs wug0[:, 0:WQ] + xT[:, 0:XQ]:
            # dispatch those two leading slices before everything else ----
            WH0 = KT * 128
            WQ0 = WH0 // 4
            wug0 = wpool.tile([128, 2 * KT * 128], bf16, tag="wug", bufs=3)
            nc.sync.dma_start(out=wug0[:, 0:WQ0], in_=d_wug[0][:, 0:WQ0])
            xT_sb = cpool.tile([128, KT * TC], bf16, name="xT_sb")
            XH = KT * TC // 2
            XQ = KT * TC // 8
            nc.sync.dma_start(out=xT_sb[:, 0:XQ], in_=d_xT[:, 0:XQ])
            nc.sync.dma_start(out=wug0[:, WQ0:WH0], in_=d_wug[0][:, WQ0:WH0])
            nc.sync.dma_start(out=xT_sb[:, XQ:XH], in_=d_xT[:, XQ:XH])
            nc.sync.dma_start(out=wug0[:, WH0:], in_=d_wug[0][:, WH0:])
            nc.sync.dma_start(out=xT_sb[:, XH:], in_=d_xT[:, XH:])
            gwb_sb = cpool.tile([128, KT * E], bf16, name="gwb_sb")
            gwd_sb = cpool.tile([128, KT * E], bf16, name="gwd_sb")
            eid_sb = cpool.tile([128, 1], f32, name="eid_sb")
            i8m_sb = cpool.tile([128, E], f32, name="i8m_sb")
            sel2_sb = cpool.tile([2, 256], f32, name="sel2_sb")
            A_sb = cpool.tile([128, KT * 2 * ER], bf16, name="A_sb")

            ident = cpool.tile([128, 128], f32, name="ident")
            make_identity(nc, ident)
            ones_col = cpool.tile([128, 1], f32, name="ones_col")
            nc.vector.memset(ones_col, 1.0)

            mixed = cpool.tile([128, HT * TC], bf16, name="mixed")
            ev_rows = cpool.tile([2, TC], f32, name="ev_rows")
            s_rows = cpool.tile([2, TC], f32, name="s_rows")
            crows = cpool.tile([2, TC], f32, name="crows")
            cb = cpool.tile([128, 2 * TC], bf16, name="cb")
            Mj = cpool.tile([128, 2 * TC], bf16, name="Mj")
            UA = cpool.tile([128, TC], bf16, name="UA")
            GA = cpool.tile([128, TC], bf16, name="GA")
            Ut = cpool.tile([128, 2 * TC], bf16, name="Ut")
            Gt = cpool.tile([128, 2 * TC], bf16, name="Gt")
            vt = cpool.tile([128, 2 * TC], bf16, name="vt")
            dacc = cpool.tile([128, 2], f32, name="dacc")
            UGstash = cpool.tile([128, STASH_P * 2 * TC], bf16,
                                 name="UGstash")

            WH = KT * 128             # 2048 cols per wug half (up | gate)

            def load_wug(i):
                t = wpool.tile([128, 2 * KT * 128], bf16, tag="wug", bufs=3)
                nc.sync.dma_start(out=t[:, 0:WH], in_=d_wug[i][:, 0:WH])
                nc.sync.dma_start(out=t[:, WH:], in_=d_wug[i][:, WH:])
                return t

            def base_chain(i, wugh):
                psUG = pspool.tile([128, 2 * TC], f32, tag="psUG", bufs=2,
                                   name="psUG")
                for k in range(KT):
                    nc.tensor.matmul(psUG[:, 0:TC],
                                     wugh[:, k * 128:(k + 1) * 128],
                                     xT_sb[:, k * TC:(k + 1) * TC],
                                     start=(k == 0), stop=(k == KT - 1))
                for k in range(KT):
                    nc.tensor.matmul(psUG[:, TC:2 * TC],
                                     wugh[:, (KT + k) * 128:(KT + k + 1) * 128],
                                     xT_sb[:, k * TC:(k + 1) * TC],
                                     start=(k == 0), stop=(k == KT - 1))
                return psUG

            # ---- pass A: base GEMMs for the first STASH_P tiles, with the
            # router token stream's DMAs interleaved into the queue (one
            # k-slice per pass-A tile, so wu/wg and xL/xR share bandwidth
            # and the first base chain isn't queued behind the full 5.6 MB
            # router stream) ----
            xd_sb = cpool.tile([128, KT * TC], bf16, name="xd_sb")
            xr_sb = cpool.tile([128, KT * SR], bf16, name="xr_sb")
            for i in range(STASH_P):
                wugh = wug0 if i == 0 else load_wug(i)
                if i == 1:
                    nc.sync.dma_start(out=gwb_sb[:], in_=d_gwb[:])
                    nc.sync.dma_start(out=gwd_sb[:], in_=d_gwd[:])
                elif i == 3:
                    nc.sync.dma_start(out=xd_sb[:], in_=d_xD[:])
                elif i == 5:
                    nc.sync.dma_start(out=xr_sb[:, 0:KT * SR // 2],
                                      in_=d_xR[:, 0:KT * SR // 2])
                elif i == 8:
                    nc.sync.dma_start(out=xr_sb[:, KT * SR // 2:],
                                      in_=d_xR[:, KT * SR // 2:])
                elif i == 11:
                    nc.sync.dma_start(out=A_sb[:], in_=d_A[:])
                elif i == 13:
                    nc.sync.dma_start(out=eid_sb[:], in_=d_eid[:])
                    nc.sync.dma_start(out=i8m_sb[:], in_=d_i8m[:])
                    nc.sync.dma_start(out=sel2_sb[:], in_=d_sel2[:])
                psUG = base_chain(i, wugh)
                nc.scalar.copy(
                    UGstash[:, i * 2 * TC:(i + 1) * 2 * TC], psUG[:])


            # ---- router matmuls (xL/xR fully streamed by now) ----
            # psL pairs [128,512]: two token-tiles' [128,8] logit blocks at
            # cols 0 and 256. ONE start per bank (a second start=True would
            # clear the whole bank's has_written and break the sibling
            # region's accumulation); the h=1 region's first write lands on
            # cleared has_written and correctly overwrites.
            psLp = [pspool.tile([128, 2 * TC], f32, tag="psLO", bufs=4,
                                name=f"psLp{p}") for p in range(4)]
            for k in range(KT):
                for h in range(2):
                    # local logits, f32-exact via bf16 error-splitting:
                    # L = (xb+xd)@(gwb+gwd), dropping the ~2^-18 xd@gwd term
                    for lhs, rhs in ((xT_sb, gwb_sb), (xd_sb, gwb_sb),
                                     (xT_sb, gwd_sb)):
                        nc.tensor.matmul(
                            psLp[0][:, h * TC:h * TC + E],
                            lhs[:, k * TC + h * 128:k * TC + (h + 1) * 128],
                            rhs[:, k * E:(k + 1) * E],
                            start=(k == 0 and h == 0 and lhs is xT_sb
                                   and rhs is gwb_sb),
                            stop=(k == KT - 1 and rhs is gwd_sb),
                            skip_group_check=True)
                for p in range(1, 4):
                    for h in range(2):
                        nc.tensor.matmul(
                            psLp[p][:, h * TC:h * TC + E],
                            xr_sb[:, k * SR + (2 * p + h - 2) * 128:
                                  k * SR + (2 * p + h - 1) * 128],
                            gwb_sb[:, k * E:(k + 1) * E],
                            start=(k == 0 and h == 0), stop=(k == KT - 1),
                            skip_group_check=(h == 1))

            for tt in range(2 * 4):
                psL = psLp[tt // 2][:, (tt % 2) * TC:(tt % 2) * TC + E]
                L = spool.tile([128, E], f32, tag="L", bufs=3)
                nc.vector.tensor_copy(L[:], psL)
                mx1 = spool.tile([128, 1], f32, tag="mx1")
                nc.vector.tensor_reduce(mx1[:], L[:], mybir.AxisListType.X,
                                        AL.max)
                msk = spool.tile([128, E], f32, tag="msk")
                nc.vector.tensor_scalar(msk[:], L[:], mx1[:], None,
                                        AL.is_equal)
                evals = spool.tile([128, 2], f32, tag="evals", bufs=3)
                nc.scalar.activation(evals[:, 0:1], mx1[:], AF.Exp)
                # mask out slot-0 winner, find second max
                big = spool.tile([128, E], f32, tag="big")
                nc.vector.tensor_scalar(big[:], msk[:], 1e30, None, AL.mult)
                L2 = spool.tile([128, E], f32, tag="L2")
                nc.vector.tensor_tensor(L2[:], L[:], big[:], AL.subtract)
                mx2 = spool.tile([128, 1], f32, tag="mx2")
                nc.vector.tensor_reduce(mx2[:], L2[:], mybir.AxisListType.X,
                                        AL.max)
                nc.scalar.activation(evals[:, 1:2], mx2[:], AF.Exp)
                # denominator accumulation (all 8 tiles)
                if tt == 0:
                    nc.vector.tensor_copy(dacc[:], evals[:])
                else:
                    nc.vector.tensor_tensor(dacc[:], dacc[:], evals[:],
                                            AL.add)
                if tt < 2:
                    # local tokens: need expert ids + value rows
                    svals = spool.tile([128, 2], f32, tag="svals")
                    mi = spool.tile([128, E], f32, tag="mi")
                    nc.vector.tensor_tensor(mi[:], msk[:], i8m_sb[:], AL.mult)
                    nc.vector.tensor_reduce(svals[:, 0:1], mi[:],
                                            mybir.AxisListType.X, AL.max)
                    msk2 = spool.tile([128, E], f32, tag="msk2")
                    nc.vector.tensor_scalar(msk2[:], L2[:], mx2[:], None,
                                            AL.is_equal)
                    mi2 = spool.tile([128, E], f32, tag="mi2")
                    nc.vector.tensor_tensor(mi2[:], msk2[:], i8m_sb[:],
                                            AL.mult)
                    nc.vector.tensor_reduce(svals[:, 1:2], mi2[:],
                                            mybir.AxisListType.X, AL.max)
                    # transpose evals/svals -> rows
                    psT = pspool.tile([2, 128], f32, tag="ps_small",
                                      name="psT")
                    nc.tensor.transpose(psT[:], evals[:], ident[:])
                    nc.vector.tensor_copy(
                        ev_rows[:, tt * 128:(tt + 1) * 128], psT[:])
                    psT2 = pspool.tile([2, 128], f32, tag="ps_small",
                                       name="psT2")
                    nc.tensor.transpose(psT2[:], svals[:], ident[:])
                    nc.vector.tensor_copy(
                        s_rows[:, tt * 128:(tt + 1) * 128], psT2[:])

            # denominators: [2,1] = dacc.T @ ones, reciprocal
            psDC = pspool.tile([2, 1], f32, tag="ps_small", name="psDC")
            nc.tensor.matmul(psDC[:], dacc[:], ones_col[:],
                             start=True, stop=True)
            rcp = cpool.tile([2, 1], f32, name="rcp")
            nc.vector.reciprocal(rcp[:], psDC[:])
            # normalized routing weights as rows [2, TC]
            nc.vector.tensor_scalar(crows[:], ev_rows[:], rcp[:], None,
                                    AL.mult)

            # broadcast slot rows along partitions via K=2 matmul with a
            # row-selector constant (sel2[:, j*128:(j+1)*128] has row j = 1)
            for j in range(2):
                psBr = pspool.tile([128, TC], f32, tag="ps_small",
                                   name="psBr")
                nc.tensor.matmul(psBr[:], sel2_sb[:, j * 128:(j + 1) * 128],
                                 crows[:], start=True, stop=True)
                nc.vector.tensor_copy(cb[:, j * TC:(j + 1) * TC], psBr[:])
                psM = pspool.tile([128, TC], f32, tag="ps_small",
                                  name="psM")
                nc.tensor.matmul(psM[:], sel2_sb[:, j * 128:(j + 1) * 128],
                                 s_rows[:], start=True, stop=True)
                nc.vector.tensor_scalar(Mj[:, j * TC:(j + 1) * TC], psM[:],
                                        eid_sb[:], None, AL.is_equal)

            # ---- A-proj (needs only xT+A; results used post-router) ----
            psUA = pspool.tile([128, TC], f32, tag="psUG", bufs=2,
                               name="psUA")
            for k in range(KT):
                nc.tensor.matmul(psUA[:],
                                 A_sb[:, k * 2 * ER: k * 2 * ER + ER],
                                 xT_sb[:, k * TC:(k + 1) * TC],
                                 start=(k == 0), stop=(k == KT - 1))
            nc.vector.tensor_copy(UA[:], psUA[:])
            psGA = pspool.tile([128, TC], f32, tag="psUG", bufs=2,
                               name="psGA")
            for k in range(KT):
                nc.tensor.matmul(psGA[:],
                                 A_sb[:, k * 2 * ER + ER:(k + 1) * 2 * ER],
                                 xT_sb[:, k * TC:(k + 1) * TC],
                                 start=(k == 0), stop=(k == KT - 1))
            nc.vector.tensor_copy(GA[:], psGA[:])

            for j in range(2):
                nc.vector.tensor_tensor(Ut[:, j * TC:(j + 1) * TC], UA[:],
                                        Mj[:, j * TC:(j + 1) * TC], AL.mult)
                nc.vector.tensor_tensor(Gt[:, j * TC:(j + 1) * TC], GA[:],
                                        Mj[:, j * TC:(j + 1) * TC], AL.mult)

            # resident LoRA B tensors: loaded mid-kernel (off the hot front)
            uB_sb = cpool.tile([128, HT * 128], bf16, name="uB_sb")
            nc.sync.dma_start(out=uB_sb[:], in_=d_uB[:])
            gB_sb = cpool.tile([128, HT * 128], bf16, name="gB_sb")
            nc.sync.dma_start(out=gB_sb[:], in_=d_gB[:])
            dA_sb = cpool.tile([128, HT * 128], bf16, name="dA_sb")
            nc.sync.dma_start(out=dA_sb[:], in_=d_dA[:])

            # ---- merged loop: full tiles STASH_P..HT-1, each also carrying
            # one stashed tile's deferred LoRA + elementwise ----
            psV = pspool.tile([128, 2 * TC], f32, tag="psV", name="psV")
            pend = []                   # [(tile_idx, ch_pair), ...]
            vstate = {"first": True}

            def flush_pend(final=False):
                for n, (pi, pch) in enumerate(pend):
                    last = final and n == len(pend) - 1
                    nc.tensor.matmul(psV[:],
                                     dA_sb[:, pi * 128:(pi + 1) * 128],
                                     pch[:], start=vstate["first"],
                                     stop=last, skip_group_check=True)
                    vstate["first"] = False
                pend.clear()

            def lora_elemwise(i, ug_src):
                ch_pair = spool.tile([128, 2 * TC], bf16, tag="chp", bufs=4)
                for j in range(2):
                    # psLO_j = [lo_up_j | lo_gate_j], one PSUM bank
                    psLO = pspool.tile([128, 2 * TC], f32, tag="psLO",
                                       bufs=4, name="psLO")
                    nc.tensor.matmul(psLO[:, 0:TC],
                                     uB_sb[:, i * 128:(i + 1) * 128],
                                     Ut[:, j * TC:(j + 1) * TC],
                                     start=True, stop=True)
                    nc.tensor.matmul(psLO[:, TC:2 * TC],
                                     gB_sb[:, i * 128:(i + 1) * 128],
                                     Gt[:, j * TC:(j + 1) * TC],
                                     start=True, stop=True)
                    # [U+lo_u | G+lo_g] in one 512-wide add (DVE reads only
                    # ONE PSUM input per op, so U|G must come from SBUF)
                    tusg = spool.tile([128, 2 * TC], bf16, tag="tusg",
                                      bufs=3)
                    nc.vector.tensor_tensor(tusg[:], ug_src, psLO[:],
                                            AL.add)
                    su = spool.tile([128, TC], bf16, tag="su")
                    nc.scalar.activation(su[:], tusg[:, 0:TC], AF.Silu)
                    hh = spool.tile([128, TC], bf16, tag="hh")
                    nc.vector.tensor_tensor(hh[:], su[:], tusg[:, TC:2 * TC],
                                            AL.mult)
                    nc.vector.tensor_tensor(ch_pair[:, j * TC:(j + 1) * TC],
                                            hh[:],
                                            cb[:, j * TC:(j + 1) * TC],
                                            AL.mult)
                nc.gpsimd.tensor_tensor(mixed[:, i * TC:(i + 1) * TC],
                                        ch_pair[:, 0:TC],
                                        ch_pair[:, TC:2 * TC], AL.add)
                pend.append((i, ch_pair))

            wd_pre = {}

            HW2 = HT * 128 // 2       # 2816 cols per wd half-tile

            def load_wd(di):
                halves = []
                for hf in range(2):
                    t = wpool.tile([128, HW2], bf16, tag="wd", bufs=4)
                    nc.sync.dma_start(
                        out=t[:], in_=d_wd[di][:, hf * HW2:(hf + 1) * HW2])
                    halves.append(t)
                return halves

            for i in range(STASH_P, HT):
                wugh = load_wug(i)
                if i == HT - 1:         # prefetch first down-proj tile
                    wd_pre[0] = load_wd(0)
                psUG = base_chain(i, wugh)
                flush_pend()
                UG_sb = spool.tile([128, 2 * TC], bf16, tag="UG", bufs=3)
                nc.scalar.copy(UG_sb[:], psUG[:])
                lora_elemwise(i, UG_sb[:])
                si = i - STASH_P
                if si < STASH_P:
                    lora_elemwise(
                        si, UGstash[:, si * 2 * TC:(si + 1) * 2 * TC])
            flush_pend(final=True)
            # masked v, slots summed: dB@vt0 + dB@vt1 == dB@(vt0+vt1)
            nc.vector.tensor_tensor(vt[:], psV[:], Mj[:], AL.mult)
            vts = cpool.tile([128, TC], bf16, name="vts")
            nc.vector.tensor_tensor(vts[:], vt[:, 0:TC], vt[:, TC:2 * TC],
                                    AL.add)

            # ---- down GEMM + LoRA-down ----
            dB_sb = cpool.tile([128, D], bf16, name="dB_sb")
            nc.sync.dma_start(out=dB_sb[:], in_=d_dB[:])
            for di in range(DT):
                wd_t = wd_pre.pop(di) if di in wd_pre else load_wd(di)
                psO = pspool.tile([128, TC], f32, tag="psUG", bufs=2,
                                  name="psO")
                for hk in range(HT):
                    w2 = wd_t[hk // 22]
                    nc.tensor.matmul(psO[:],
                                     w2[:, (hk % 22) * 128:(hk % 22 + 1) * 128],
                                     mixed[:, hk * TC:(hk + 1) * TC],
                                     start=(hk == 0), stop=False,
                                     skip_group_check=True)
                nc.tensor.matmul(psO[:], dB_sb[:, di * 128:(di + 1) * 128],
                                 vts[:], start=False, stop=True,
                                 skip_group_check=True)
                o_sb = spool.tile([128, TC], f32, tag="o_sb")
                nc.scalar.copy(o_sb[:], psO[:])
                nc.sync.dma_start(out=d_out[di * 128:(di + 1) * 128, :],
                                  in_=o_sb[:])

    nc.compile()
    return nc


def _prep_shared(inputs):
    """Host-side layout prep of weight tensors (shared across cores)."""
    import ml_dtypes
    bf16 = np.dtype(ml_dtypes.bfloat16)
    f32 = np.float32

    def c(a, dt):
        return np.ascontiguousarray(a.astype(dt, copy=False))

    w_up, w_gate, w_down = inputs["w_up"], inputs["w_gate"], inputs["w_down"]
    wu = (w_up.reshape(HT, 128, KT, 128).transpose(0, 3, 2, 1)
          .reshape(HT, 128, KT * 128))
    wg = (w_gate.reshape(HT, 128, KT, 128).transpose(0, 3, 2, 1)
          .reshape(HT, 128, KT * 128))
    wug = c(np.concatenate([wu, wg], axis=2), bf16)
    wd = c(w_down.reshape(DT, 128, HT, 128).transpose(0, 3, 2, 1)
           .reshape(DT, 128, HT * 128), bf16)

    A_stack = np.concatenate([
        inputs["up_A"].reshape(ER, D),
        inputs["gate_A"].reshape(ER, D)], axis=0)          # [2*ER, D]
    # Ah[p, k*2ER + m] = A_stack[m, k*128+p]
    Ah = c(A_stack.reshape(2 * ER, KT, 128).transpose(2, 1, 0)
           .reshape(128, KT * 2 * ER), bf16)

    # resident B tensors: [er=128 partitions, h cols] row-major
    up_B_all = (inputs["up_B"].transpose(0, 2, 1).reshape(ER, H)
                * ALPHA).astype(f32)
    gate_B_all = (inputs["gate_B"].transpose(0, 2, 1).reshape(ER, H)
                  * ALPHA).astype(f32)
    uB = c(up_B_all, bf16)                                 # [128, H]
    gB = c(gate_B_all, bf16)
    down_A_all = inputs["down_A"].reshape(ER, H).astype(f32)
    # dA[p, i*128+er] = down_A[er, i*128+p]  (lhsT [h-part, er-free])
    dA = c(down_A_all.T.reshape(HT, 128, ER).transpose(1, 0, 2)
           .reshape(128, HT * ER), bf16)
    down_B_all = (inputs["down_B"].transpose(0, 2, 1).reshape(ER, D)
                  * ALPHA).astype(f32)
    dB = c(down_B_all, bf16)

    gate_wT = inputs["gate_w"].T.astype(f32)               # [D, E]
    gw = c(gate_wT.reshape(KT, 128, E).transpose(1, 0, 2)
           .reshape(128, KT * E), f32)
    gwb = c(gw, bf16)
    gwd = c(gw - gwb.astype(f32), bf16)

    eid = (8.0 - (np.arange(128) // R)).astype(f32).reshape(128, 1)
    i8m = np.tile((8.0 - np.arange(E)).astype(f32), (128, 1))
    sel2 = np.zeros((2, 256), f32)
    sel2[0, 0:128] = 1.0
    sel2[1, 128:256] = 1.0

    return dict(wug=wug, wd=wd, Ah=Ah, uB=uB, gB=gB, dA=dA, dB=dB,
                gwb=gwb, gwd=gwd, eid=eid, i8m=i8m, sel2=sel2)


def _in_maps(inputs):
    """Build per-core input maps (shared weights + per-core x slices)."""
    import ml_dtypes
    bf16 = np.dtype(ml_dtypes.bfloat16)
    shared = _prep_shared(inputs)
    x = np.asarray(inputs["x"]).astype(np.float32)
    xt = x.reshape(T, D)

    maps = []
    for cix in range(NCORES):
        b = (cix * TC) // S
        o = (cix * TC) % S                                 # offset in batch
        xb = xt[b * S:(b + 1) * S]                         # [S, D] batch
        rolled = np.concatenate([xb[o:], xb[:o]], axis=0)  # local 256 first
        m = dict(shared)
        # prepacked SBUF layouts: [128, k*W + t] = x.T[k*128+p, t]
        loc = np.ascontiguousarray(
            rolled[0:TC].T.reshape(KT, 128, TC).transpose(1, 0, 2)
            .reshape(128, KT * TC))                        # f32
        xb = loc.astype(bf16)
        m["xT"] = xb
        m["xD"] = np.ascontiguousarray((loc - xb.astype(np.float32))
                                       .astype(bf16))
        m["xR"] = np.ascontiguousarray(
            rolled[TC:].T.astype(bf16).reshape(KT, 128, SR)
            .transpose(1, 0, 2).reshape(128, KT * SR))
        maps.append(m)
    return maps


def kernel(**inputs):
    from concourse.bass_utils import run_bass_kernel_spmd

    inputs = {k: np.asarray(v) for k, v in inputs.items()}
    if "nc" not in _cache:
        _cache["nc"] = _build()
    nc = _cache["nc"]

    in_maps = _in_maps(inputs)
    res = run_bass_kernel_spmd(nc, in_maps, list(range(NCORES)))
    out = np.empty((T, D), np.float32)
    for cix in range(NCORES):
        out[cix * TC:(cix + 1) * TC, :] = res.results[cix]["outT"].T
    return out.reshape(B, S, D)
